# revision 1
# baseline (speedup 1.0000x reference)
"""Trainium2 Bass kernel for nn_FraudDetectionModel (temporal encoder + 2-layer
GAT + classifier). Self-contained: hardcodes shapes, shards across 8 cores.

Sharding: nodes data-parallel (12500/core, degree-sorted, tiles of 128 lanes);
edges partitioned by destination core into padded slot grids; per-edge source
data fetched with GPSIMD dma_gather from packed HBM tables; the GAT2 node
table is shared across cores via AllGather.
"""
import numpy as np
import ml_dtypes

bf16 = ml_dtypes.bfloat16
NEG = -1.0e9


class _Cfg:
    def __init__(self, n=100000, e=1600000, ncore=8):
        self.N, self.E, self.NCORE = n, e, ncore
        self.T, self.F = 50, 10
        self.H1, self.C1, self.LAT = 4, 32, 64
        self.L = n // ncore
        self.TILES = (self.L + 127) // 128
        self.LP = self.TILES * 128
        self.GG = 1024
        self.NGC = (n + self.GG - 1) // self.GG
        self.GN = self.NGC * self.GG
        # x table: 4 nodes (16 bf16 each) per row, 128B data @ 256B stride
        self.NR1 = self.GN // 4 + 1          # +1 pad row
        self.PR1 = self.GN // 4              # pad row id
        # g2 table: 2 nodes (68 bf16 each) per row, 272B data @ 512B stride
        self.NR2 = (ncore * self.LP) // 2
        # temporal K-tiling
        self.TQ = [11, 11, 11, 11, 6]
        self.TQOFF = [0, 11, 22, 33, 44]
        self.KQ = [121, 121, 121, 121, 66]
        self.QW = [704, 704, 704, 704, 384]   # matmul cols per q (64*tq)
        # collective chunking (tile rows, multiples of 128, even node counts)
        nck = 4
        per = ((self.TILES + nck - 1) // nck)
        self.CCH = []
        t = 0
        while t < self.TILES:
            nt = min(per, self.TILES - t)
            self.CCH.append((t * 128, (t + nt) * 128))
            t += nt

    def cchunks(self):
        """(r0, r1, base) rows of the allgathered g2 table."""
        out = []
        base = 0
        for r0, r1 in self.CCH:
            out.append((r0, r1, base))
            base += self.NCORE * (r1 - r0)
        return out


CFG = _Cfg()


def _row1_of(cfg, g):
    g = np.asarray(g)
    q, r = g // cfg.GG, g % cfg.GG
    return q * cfg.GG + (r % 128) * 8 + r // 128


# ======================================================================
# host prep: graph
# ======================================================================
def _prep_graph(cfg, edge_index):
    src = edge_index[0].astype(np.int64)
    dst = edge_index[1].astype(np.int64)
    loops = np.arange(cfg.N, dtype=np.int64)
    src = np.concatenate([src, loops])
    dst = np.concatenate([dst, loops])
    core = dst // cfg.L
    percore, invs = [], []
    for c in range(cfg.NCORE):
        m = core == c
        es, ed = src[m], dst[m] - c * cfg.L
        deg = np.bincount(ed, minlength=cfg.L)
        perm = np.argsort(deg, kind="stable")
        inv = np.empty(cfg.L, np.int64)
        inv[perm] = np.arange(cfg.L)
        degp = np.zeros(cfg.LP, np.int64)
        degp[:cfg.L] = deg[perm]
        pos = inv[ed]
        order = np.argsort(pos, kind="stable")
        percore.append((es[order], pos[order], degp, perm))
        invs.append(inv)
    D = np.ones(cfg.TILES, np.int64)
    for es, pos, degp, perm in percore:
        D = np.maximum(D, degp.reshape(cfg.TILES, 128).max(1))
    return percore, invs, D


def _chunk_sched(cfg, D, cap_slots=128, max_tiles=6):
    chunks, t = [], 0
    while t < cfg.TILES:
        nt = 1
        while (t + nt < cfg.TILES and nt < max_tiles
               and int(max(D[t:t + nt + 1])) * (nt + 1) <= cap_slots):
            nt += 1
        chunks.append((t, nt, int(max(D[t:t + nt]))))
        t += nt
    offs = np.concatenate([[0], np.cumsum([nt * dc for _, nt, dc in chunks])])
    return chunks, offs.astype(np.int64)


def _row2_of(cfg, g, invs, cch):
    g = np.asarray(g)
    owner = g // cfg.L
    r = np.empty_like(g)
    for c in range(cfg.NCORE):
        m = owner == c
        if m.any():
            r[m] = invs[c][g[m] - c * cfg.L]
    out = np.empty_like(g)
    for (r0, r1, base) in cch:
        m = (r >= r0) & (r < r1)
        out[m] = base + owner[m] * (r1 - r0) + (r[m] - r0)
    return out


def _wrap16(lin):
    n = len(lin)
    assert n % 16 == 0
    w = np.zeros((16, n // 16), np.int16)
    w[np.arange(n) % 16, np.arange(n) // 16] = lin
    return np.tile(w, (8, 1))


def _grids(cfg, percore_c, invs, D, chunks, offs, cch):
    """Per-core slot data in chunk-padded layout. Returns:
       idx1w [128, S] int16 (wrapped, x-table quad rows, pads -> PR1)
       msel4 [128, S*4] bf16 (quarter one-hot; pads -> quarter of pad row)
       idx2wA/idx2wB [128, S] int16 (pair rows, value-split, -1 skip)
       nvalidA/B per chunk (list of int)
       mhalf [128, S*2] bf16 (pair-half one-hot x validity)
    """
    es, pos, degp, perm = percore_c
    S = int(offs[-1])
    start = np.concatenate([[0], np.cumsum(degp)])[:-1]
    rr = np.arange(len(pos)) - start[pos]
    tile_of = pos // 128
    lane = pos % 128
    # chunk/col of each tile
    tile2chunk = np.zeros(cfg.TILES, np.int64)
    tile2col = np.zeros(cfg.TILES, np.int64)
    for ci, (t0, nt, dc) in enumerate(chunks):
        tile2chunk[t0:t0 + nt] = ci
        tile2col[t0:t0 + nt] = offs[ci] + (np.arange(nt)) * dc
    col = tile2col[tile_of] + rr
    flat = lane * S + col

    row1 = _row1_of(cfg, es)
    idx1 = np.full(128 * S, cfg.PR1 * 4, np.int64)   # quarter 0 of pad row
    idx1[flat] = row1
    row2 = _row2_of(cfg, es, invs, cch)
    idx2 = np.full(128 * S, -1, np.int64)
    idx2[flat] = row2
    # dummy lanes: slot 0 -> node 0 (real row) so denominators stay finite
    dummy = np.nonzero(degp == 0)[0]
    dl, dt = dummy % 128, dummy // 128
    dflat = dl * S + tile2col[dt]
    idx1[dflat] = int(_row1_of(cfg, np.int64(0)))
    idx2[dflat] = int(_row2_of(cfg, np.zeros(1, np.int64), invs, cch)[0])

    idx1 = idx1.reshape(128, S)
    idx2 = idx2.reshape(128, S)
    q1 = (idx1 % 4).astype(np.int64)
    msel4 = np.zeros((128, S, 4), np.float32)
    np.put_along_axis(msel4, q1[:, :, None], 1.0, axis=2)
    h2 = (idx2 % 2).clip(0).astype(np.int64)
    mhalf = np.zeros((128, S, 2), np.float32)
    np.put_along_axis(mhalf, h2[:, :, None], 1.0, axis=2)
    mhalf[idx2 < 0] = 0.0

    pair = idx2 // 2
    # wrapped streams per chunk, slot-major (i = col*128 + lane)
    i1 = np.empty(0, np.int64)
    i2a = np.empty(0, np.int64)
    i2b = np.empty(0, np.int64)
    nvA, nvB = [], []
    SPL = 32768
    for ci, (t0, nt, dc) in enumerate(chunks):
        cs = slice(int(offs[ci]), int(offs[ci + 1]))
        lin1 = (idx1[:, cs] // 4).T.reshape(-1)       # [W, 128] -> i=col*128+p
        pc = pair[:, cs].T.reshape(-1)
        la = np.where((pc >= 0) & (pc < SPL), pc, -1)
        lb = np.where(pc >= SPL, pc - SPL, -1)
        # pads (pc<0): route to call A row 0 (mhalf zero kills them)
        la = np.where(pc < 0, 0, la)
        nvA.append(int((la >= 0).sum()))
        nvB.append(int((lb >= 0).sum()))
        i1 = np.concatenate([i1, lin1])
        i2a = np.concatenate([i2a, la])
        i2b = np.concatenate([i2b, lb])
    return (_wrap16(i1), msel4.reshape(128, S * 4).astype(bf16),
            _wrap16(i2a), _wrap16(i2b), nvA, nvB,
            mhalf.reshape(128, S * 2).astype(bf16))


# ======================================================================
# host prep: packing
# ======================================================================
def _pack_global_x(cfg, x):
    xp = np.zeros((cfg.GN, cfg.F), np.float32)
    xp[:cfg.N] = x
    v = xp.reshape(cfg.NGC, 8, 128, cfg.F)
    return v.transpose(0, 1, 3, 2).reshape(cfg.NGC * 8 * cfg.F, 128).astype(bf16)


def _pack_perm_x(cfg, x, perm, c):
    xp = np.zeros((cfg.LP, cfg.F), np.float32)
    xp[:cfg.L] = x[c * cfg.L + perm]
    rows = []
    g = 0
    while g * cfg.GG < cfg.LP:
        n0 = g * cfg.GG
        nt = min(8, (cfg.LP - n0) // 128)
        v = xp[n0:n0 + nt * 128].reshape(nt, 128, cfg.F)
        rows.append(v.transpose(0, 2, 1).reshape(nt * cfg.F, 128))
        g += 1
    return np.concatenate(rows, 0).astype(bf16)


def _a1d_groups(cfg):
    """(n_tiles, k) per dense a1d group."""
    out = []
    g = 0
    while g * cfg.GG < cfg.LP:
        nt = min(8, (cfg.LP - g * cfg.GG) // 128)
        out.append(nt)
        g += 1
    return out


def _pack_td(cfg, td, perm, c):
    tdp = np.zeros((cfg.LP, cfg.T, cfg.F), np.float32)
    tdp[:cfg.L] = td[c * cfg.L + perm]
    out = np.zeros((cfg.TILES, 128, 640), np.float32)
    nodes = tdp.reshape(cfg.TILES, 128, cfg.T, cfg.F)
    for q in range(5):
        tq, kq = cfg.TQ[q], cfg.KQ[q]
        blk = nodes[:, :, cfg.TQOFF[q]:cfg.TQOFF[q] + tq, :]
        r = np.zeros((cfg.TILES, tq, 11, 128), np.float32)
        r[:, :, :cfg.F] = blk.transpose(0, 2, 3, 1)
        r[:, :, cfg.F] = 1.0
        out[:, :kq, 128 * q:128 * q + 128] = r.reshape(cfg.TILES, kq, 128)
    return np.ascontiguousarray(out).astype(bf16)


def _block_diag(b, n):
    r, c = b.shape
    out = np.zeros((n * r, n * c), np.float32)
    for i in range(n):
        out[i * r:(i + 1) * r, i * c:(i + 1) * c] = b
    return out


def _prep_weights(cfg, w):
    F, H1, C1, LAT = cfg.F, cfg.H1, cfg.C1, cfg.LAT
    gW1 = w["gW1"].astype(np.float32)
    A1s = (gW1.reshape(F, H1, C1) * w["ga1_src"][None]).sum(-1)
    A1d = (gW1.reshape(F, H1, C1) * w["ga1_dst"][None]).sum(-1)
    gW2 = w["gW2"].astype(np.float32)
    A2s = (gW2.reshape(H1 * C1, 1, LAT) * w["ga2_src"][None]).sum(-1)[:, 0]
    A2d = (gW2.reshape(H1 * C1, 1, LAT) * w["ga2_dst"][None]).sum(-1)[:, 0]

    B = np.zeros((F, 16), np.float32)
    B[:, :F] = np.eye(F)
    B[:, F:F + H1] = A1s
    con = {}
    con["rhs_xaug"] = _block_diag(B, 8)                       # [80, 128]
    con["rhs_a1d"] = _block_diag(A1d, 8)                      # [80, 32]
    ag = _a1d_groups(cfg)
    if ag and ag[-1] != 8:
        con["rhs_a1d_p"] = _block_diag(A1d, ag[-1])
    W1b = np.concatenate([w["tW1"], w["tb1"][None]], 0).astype(np.float32)
    con["rhs_mm1f"] = np.concatenate(
        [_block_diag(W1b, 11), np.tile(W1b, (11, 1))], 1)     # [121, 768]
    con["rhs_mm1p"] = np.concatenate(
        [_block_diag(W1b, 6), np.tile(W1b, (6, 1))], 1)       # [66, 448]

    rhs_g1 = np.zeros((4 * F, H1 * C1), np.float32)
    for h in range(H1):
        rhs_g1[h * F:(h + 1) * F, h * C1:(h + 1) * C1] = gW1[:, h * C1:(h + 1) * C1]
    con["rhs_g1"] = rhs_g1                                    # [40, 128] f32
    con["gb1bc"] = np.tile(w["gb1"][None], (128, 1))          # [128, 128] f32

    gw2ext = np.zeros((H1 * C1, 68), np.float32)
    gw2ext[:, :LAT] = gW2
    gw2ext[:, LAT] = A2s
    gw2ext[:, LAT + 1] = A2d
    con["gw2ext"] = gw2ext                                    # [128, 68] f32

    ones = np.ones(H1 * C1, np.float32)
    adj2 = float(-(ones @ A2s) - (ones @ A2d))
    g2shift = ones @ gW2
    cW1 = w["cW1"].astype(np.float32)
    con["cw1f"] = np.concatenate(
        [(w["tW2"] / 100.0) @ cW1[:LAT], cW1[LAT:]], 0)       # [128, 64] f32
    cb1p = (w["tb2"] @ cW1[:LAT]) + ((w["gb2"] - g2shift) @ cW1[LAT:]) + w["cb1"]
    con["cb1bc"] = np.tile(cb1p[None], (128, 1))              # [128, 64] f32
    con["cw2bc"] = np.tile(w["cW2"][:, 0][None], (128, 1))    # [128, 64] f32
    padrow = np.zeros((1, 128), np.float32)
    for qq in range(4):
        padrow[0, 16 * qq + F:16 * qq + F + H1] = NEG
    con["padrow"] = padrow
    return con, adj2, float(w["cb2"][0])


# ======================================================================
# host: per-edge input grids (halo exchange done host-side)
# ======================================================================
def _edge_layout(cfg, percore_c, D, chunks, offs):
    """Slot layout for one core: returns (flat slot index per edge, S)."""
    es, pos, degp, perm = percore_c
    S = int(offs[-1])
    start = np.concatenate([[0], np.cumsum(degp)])[:-1]
    rr = np.arange(len(pos)) - start[pos]
    tile_of = pos // 128
    lane = pos % 128
    tile2col = np.zeros(cfg.TILES, np.int64)
    for ci, (t0, nt, dc) in enumerate(chunks):
        tile2col[t0:t0 + nt] = offs[ci] + np.arange(nt) * dc
    col = tile2col[tile_of] + rr
    return lane * S + col, S


def _xe_grid(cfg, x, percore_c, flat, S):
    """xe grid [128, S*16] bf16: [x(10), -1e9 pad marker, 0...]."""
    es = percore_c[0]
    g = np.zeros((128 * S, 16), np.float32)
    g[:, cfg.F] = NEG                    # default: pad marker
    g[flat, :cfg.F] = x[es]
    g[flat, cfg.F] = 0.0
    # dummy-lane slot 0: real node 0 so softmax denominators stay finite
    degp = percore_c[2]
    dummy = np.nonzero(degp == 0)[0]
    if len(dummy):
        chunks_col = np.nonzero(g.reshape(128, S, 16)[0, :, 0] * 0 == 0)[0]  # noqa
    return g.reshape(128, S * 16).astype(bf16)


def _xe_fix_dummies(cfg, grid, percore_c, chunks, offs, x):
    degp = percore_c[2]
    dummy = np.nonzero(degp == 0)[0]
    if not len(dummy):
        return grid
    tile2col = np.zeros(cfg.TILES, np.int64)
    for ci, (t0, nt, dc) in enumerate(chunks):
        tile2col[t0:t0 + nt] = offs[ci] + np.arange(nt) * dc
    g = grid.reshape(128, -1, 16)
    lanes, tiles = dummy % 128, dummy // 128
    cols = tile2col[tiles]
    g[lanes, cols, :cfg.F] = x[0].astype(bf16)
    g[lanes, cols, cfg.F] = 0.0
    return grid


def _g2e_grid(cfg, g2cat, percore_c, invs, cch, flat, S):
    """g2e grid [128, S*72] bf16 from allgathered g2 table (host gather)."""
    es = percore_c[0]
    row2 = _row2_of(cfg, es, invs, cch)
    g = np.zeros((128 * S, 72), bf16)
    g[:, 64] = bf16(NEG)                 # pad marker in a2s column
    g[flat, :68] = g2cat[row2]
    degp = percore_c[2]
    dummy = np.nonzero(degp == 0)[0]
    return g.reshape(128, S * 72), dummy


def _g2e_fix_dummies(cfg, grid, percore_c, chunks, offs, g2cat, invs, cch):
    degp = percore_c[2]
    dummy = np.nonzero(degp == 0)[0]
    if not len(dummy):
        return grid
    tile2col = np.zeros(cfg.TILES, np.int64)
    for ci, (t0, nt, dc) in enumerate(chunks):
        tile2col[t0:t0 + nt] = offs[ci] + np.arange(nt) * dc
    g = grid.reshape(128, -1, 72)
    lanes, tiles = dummy % 128, dummy // 128
    cols = tile2col[tiles]
    r0 = int(_row2_of(cfg, np.zeros(1, np.int64), invs, cch)[0])
    g[lanes, cols, :68] = g2cat[r0]
    return grid


# ======================================================================
# device builders
# ======================================================================
def _mk(ap_tensor, offset, dims):
    from concourse.bass import AP
    return AP(ap_tensor, int(offset), [list(d) for d in dims])


def build_exec1(cfg, con, adj2, chunks, offs):
    """Program 1: temporal encoder (s_pre), a1d, GAT1 -> g2/a2s table rows,
    a2d. Inputs: td_packed, xp_pack, xe_grid, consts."""
    import concourse.bacc as bacc
    import concourse.mybir as mybir
    import concourse.tile as tile
    from concourse.masks import make_identity

    fp32 = mybir.dt.float32
    b16 = mybir.dt.bfloat16
    F, H1, TQ, KQ, QW = cfg.F, cfg.H1, cfg.TQ, cfg.KQ, cfg.QW
    S = int(offs[-1])
    ag = _a1d_groups(cfg)

    nc = bacc.Bacc(None, target_bir_lowering=False, debug=False)
    td = nc.dram_tensor("td", [cfg.TILES * 128, 640], b16, kind="ExternalInput")
    xp = nc.dram_tensor("xp", [sum(g * F for g in ag), 128], b16, kind="ExternalInput")
    xe = nc.dram_tensor("xe", [128, S * 16], b16, kind="ExternalInput")
    c_mm1f = nc.dram_tensor("c_mm1f", [121, 768], b16, kind="ExternalInput")
    c_mm1p = nc.dram_tensor("c_mm1p", [66, 448], b16, kind="ExternalInput")
    c_a1d = nc.dram_tensor("c_a1d", [80, 32], b16, kind="ExternalInput")
    c_a1dp = (nc.dram_tensor("c_a1dp", [ag[-1] * F, ag[-1] * 4], b16,
                             kind="ExternalInput") if ag[-1] != 8 else None)
    c_a1sp = nc.dram_tensor("c_a1sp", [128, 44], b16, kind="ExternalInput")
    c_g1 = nc.dram_tensor("c_g1", [40, 128], fp32, kind="ExternalInput")
    c_gb1 = nc.dram_tensor("c_gb1", [128, 128], fp32, kind="ExternalInput")
    c_g2e = nc.dram_tensor("c_g2e", [128, 68], fp32, kind="ExternalInput")

    o_g2 = nc.dram_tensor("o_g2", [cfg.LP, 68], b16, kind="ExternalOutput")
    o_spre = nc.dram_tensor("o_spre", [128, cfg.TILES * 64], b16, kind="ExternalOutput")
    o_a2d = nc.dram_tensor("o_a2d", [128, cfg.TILES], fp32, kind="ExternalOutput")

    with tile.TileContext(nc) as tc:
        with (
            tc.tile_pool(name="const", bufs=1) as cp,
            tc.tile_pool(name="tds", bufs=3) as tds,
            tc.tile_pool(name="qps", bufs=2, space="PSUM") as qps,
            tc.tile_pool(name="lps", bufs=2, space="PSUM") as lps,
            tc.tile_pool(name="gps", bufs=2, space="PSUM") as gps,
            tc.tile_pool(name="ev", bufs=2) as ev,
            tc.tile_pool(name="per", bufs=1) as per,
            tc.tile_pool(name="xep", bufs=2) as xep,
            tc.tile_pool(name="ed", bufs=2) as ed,
            tc.tile_pool(name="tl", bufs=3) as tl,
        ):
            ident = cp.tile([128, 128], fp32)
            make_identity(nc, ident[:])
            mm1f = cp.tile([121, 768], b16)
            nc.sync.dma_start(out=mm1f[:], in_=c_mm1f[:])
            mm1p = cp.tile([66, 448], b16)
            nc.sync.dma_start(out=mm1p[:], in_=c_mm1p[:])
            a1dW = cp.tile([80, 32], b16)
            nc.sync.dma_start(out=a1dW[:], in_=c_a1d[:])
            if c_a1dp is not None:
                a1dWp = cp.tile([ag[-1] * F, ag[-1] * 4], b16)
                nc.sync.dma_start(out=a1dWp[:], in_=c_a1dp[:])
            a1sp = cp.tile([128, 44], b16)
            nc.sync.dma_start(out=a1sp[:], in_=c_a1sp[:])
            g1W = cp.tile([40, 128], fp32)
            nc.sync.dma_start(out=g1W[:], in_=c_g1[:])
            gb1bc = cp.tile([128, 128], fp32)
            nc.sync.dma_start(out=gb1bc[:], in_=c_gb1[:])
            g2eW = cp.tile([128, 68], fp32)
            nc.sync.dma_start(out=g2eW[:], in_=c_g2e[:])

            adj_t = cp.tile([128, 1], fp32)
            nc.vector.memset(adj_t[:], float(adj2))
            spre = per.tile([128, cfg.TILES * 64], fp32)
            a1d_all = per.tile([128, cfg.TILES * 4], b16)
            a2d_all = per.tile([128, cfg.TILES], fp32)

            # ---- dense a1d ----
            for g, nt in enumerate(ag):
                k = nt * F
                lh = tl.tile([128, 128], b16, tag="a1dlh")
                nc.sync.dma_start(
                    out=lh[0:k, 0:128],
                    in_=xp[sum(x * F for x in ag[:g]):sum(x * F for x in ag[:g]) + k, :])
                ps = gps.tile([128, 128], fp32, tag="g")
                rhs = a1dW[0:k, 0:4 * nt] if nt == 8 else a1dWp[0:k, 0:4 * nt]
                nc.tensor.matmul(ps[:, 0:4 * nt], lhsT=lh[0:k, 0:128], rhs=rhs,
                                 start=True, stop=True)
                nc.vector.tensor_copy(out=a1d_all[:, 32 * g:32 * g + 4 * nt],
                                      in_=ps[:, 0:4 * nt])

            # ---- temporal ----
            for t in range(cfg.TILES):
                st = tds.tile([128, 640], b16)
                nc.sync.dma_start(out=st[:], in_=td[128 * t:128 * (t + 1), :])
                lin = lps.tile([128, 64], fp32)
                sacc = tl.tile([128, 64], fp32, tag="sacc")
                for q in range(5):
                    kq, w = KQ[q], QW[q]
                    rhs = mm1f if q < 4 else mm1p
                    ps = qps.tile([128, 704], fp32, tag="qtile")
                    lhsT = st[0:kq, 128 * q:128 * (q + 1)]
                    for c0 in range(0, w, 512):
                        c1 = min(c0 + 512, w)
                        nc.tensor.matmul(ps[:, c0:c1], lhsT=lhsT,
                                         rhs=rhs[0:kq, c0:c1], start=True, stop=True)
                    nc.tensor.matmul(lin[:], lhsT=lhsT,
                                     rhs=rhs[0:kq, QW[q]:QW[q] + 64],
                                     start=(q == 0), stop=(q == 4))
                    tq = TQ[q]
                    tgt = sacc[:] if q == 0 else None
                    if q < 2:
                        red = sacc if q == 0 else tl.tile([128, 64], fp32, tag="tred")
                        nc.vector.tensor_reduce(
                            out=red[:],
                            in_=_mk(ps.tensor, 0, [ps[:].ap[0], [1, 64], [64, tq]]),
                            axis=mybir.AxisListType.X, op=mybir.AluOpType.add,
                            apply_absolute_value=True)
                    else:
                        stg = ev.tile([128, 704], b16, tag="evs")
                        nc.scalar.activation(out=stg[:, 0:w], in_=ps[:, 0:w],
                                             func=mybir.ActivationFunctionType.Abs)
                        red = tl.tile([128, 64], fp32, tag="tred")
                        nc.vector.tensor_reduce(
                            out=red[:],
                            in_=_mk(stg.tensor, 0, [stg[:].ap[0], [1, 64], [64, tq]]),
                            axis=mybir.AxisListType.X, op=mybir.AluOpType.add)
                    if q > 0:
                        nc.vector.tensor_add(out=sacc[:], in0=sacc[:], in1=red[:])
                # s_pre = sacc + lin
                nc.vector.tensor_add(out=spre[:, 64 * t:64 * (t + 1)],
                                     in0=sacc[:], in1=lin[:])

            # ---- GAT1 per chunk ----
            for ci, (t0, nt, dc) in enumerate(chunks):
                W = nt * dc
                xeb = xep.tile([128, 128 * 16], b16, tag="xeb")
                nc.sync.dma_start(out=xeb[:, 0:W * 16],
                                  in_=xe[:, int(offs[ci]) * 16:(int(offs[ci]) + W) * 16])
                # a1s_e per head: a1s_h[t,j] = sum_k xe[t,j,k]*A1sp[h,k]
                tmpa = ed.tile([128, 128 * 11], b16, tag="tmpa")
                a1se = ed.tile([128, 128 * 4], fp32, tag="a1se")
                for h in range(4):
                    nc.vector.tensor_tensor(
                        out=tmpa[:, 0:W * 11],
                        in0=_mk(xeb.tensor, 0,
                                [xeb[:].ap[0], [16 * dc, nt], [16, dc], [1, 11]]),
                        in1=_mk(a1sp.tensor, 11 * h,
                                [a1sp[:].ap[0], [0, nt], [0, dc], [1, 11]]),
                        op=mybir.AluOpType.mult)
                    nc.vector.tensor_reduce(
                        out=a1se[:, h * W:(h + 1) * W],
                        in_=_mk(tmpa.tensor, 0,
                                [tmpa[:].ap[0], [11, W], [1, 11]]),
                        axis=mybir.AxisListType.X, op=mybir.AluOpType.add)
                # esum layout (h, t, j)
                esum = ed.tile([128, 128 * 4], b16, tag="esum")
                nc.vector.tensor_tensor(
                    out=esum[:, 0:W * 4],
                    in0=a1se[:, 0:W * 4],
                    in1=_mk(a1d_all.tensor, 4 * t0,
                            [a1d_all[:].ap[0], [1, 4], [4, nt], [0, dc]]),
                    op=mybir.AluOpType.add)
                lr = ed.tile([128, 128 * 4], b16, tag="lr")
                nc.vector.scalar_tensor_tensor(
                    out=lr[:, 0:W * 4], in0=esum[:, 0:W * 4], scalar=0.2,
                    in1=esum[:, 0:W * 4],
                    op0=mybir.AluOpType.mult, op1=mybir.AluOpType.max)
                wv = ed.tile([128, 128 * 4], b16, tag="wv")
                nc.scalar.activation(out=wv[:, 0:W * 4], in_=lr[:, 0:W * 4],
                                     func=mybir.ActivationFunctionType.Exp)
                # denom layout (h, t)
                den = ed.tile([128, 128 // 8 * 4], fp32, tag="den")
                nc.vector.tensor_reduce(
                    out=den[:, 0:nt * 4],
                    in_=_mk(wv.tensor, 0,
                            [wv[:].ap[0], [dc, 4 * nt], [1, dc]]),
                    axis=mybir.AxisListType.X, op=mybir.AluOpType.add)
                rec = ed.tile([128, 128 // 8 * 4], fp32, tag="rec")
                nc.vector.reciprocal(out=rec[:, 0:nt * 4], in_=den[:, 0:nt * 4])
                # xagg layout (t, h, k)
                tmpx = ed.tile([128, 128 * 10], b16, tag="tmpx")
                xaggr = ed.tile([128, 128 // 8 * 40], fp32, tag="xaggr")
                for h in range(4):
                    nc.vector.tensor_tensor(
                        out=tmpx[:, 0:W * 10],
                        in0=_mk(xeb.tensor, 0,
                                [xeb[:].ap[0], [16 * dc, nt], [16, dc], [1, 10]]),
                        in1=_mk(wv.tensor, h * W,
                                [wv[:].ap[0], [dc, nt], [1, dc], [0, 10]]),
                        op=mybir.AluOpType.mult)
                    nc.vector.tensor_reduce(
                        out=_mk(xaggr.tensor, 10 * h,
                                [xaggr[:].ap[0], [40, nt], [1, 10]]),
                        in_=_mk(tmpx.tensor, 0,
                                [tmpx[:].ap[0], [10 * dc, nt], [1, 10], [10, dc]]),
                        axis=mybir.AxisListType.X, op=mybir.AluOpType.add)
                xagg = ed.tile([128, 128 // 8 * 40], fp32, tag="xagg")
                nc.vector.tensor_tensor(
                    out=xagg[:, 0:nt * 40],
                    in0=xaggr[:, 0:nt * 40],
                    in1=_mk(rec.tensor, 0,
                            [rec[:].ap[0], [1, nt], [nt, 4], [0, 10]]),
                    op=mybir.AluOpType.mult)
                for ti in range(nt):
                    t = t0 + ti
                    ps1 = gps.tile([128, 128], fp32, tag="g")
                    nc.tensor.transpose(out=ps1[0:40, :],
                                        in_=xagg[:, 40 * ti:40 * (ti + 1)],
                                        identity=ident[:])
                    stag = tl.tile([40, 128], fp32, tag="stag")
                    nc.vector.tensor_copy(out=stag[:], in_=ps1[0:40, :])
                    out1 = gps.tile([128, 128], fp32, tag="g")
                    nc.tensor.matmul(out1[:], lhsT=stag[:], rhs=g1W[:],
                                     start=True, stop=True)
                    y1 = tl.tile([128, 128], fp32, tag="y1")
                    nc.vector.tensor_add(out=y1[:], in0=out1[:], in1=gb1bc[:])
                    mn = tl.tile([128, 128], fp32, tag="mn")
                    nc.vector.tensor_scalar_min(out=mn[:], in0=y1[:], scalar1=0.0)
                    ex = tl.tile([128, 128], fp32, tag="ex")
                    nc.scalar.activation(out=ex[:], in_=mn[:],
                                         func=mybir.ActivationFunctionType.Exp)
                    hs = tl.tile([128, 128], fp32, tag="hs")
                    nc.vector.scalar_tensor_tensor(
                        out=hs[:], in0=y1[:], scalar=0.0, in1=ex[:],
                        op0=mybir.AluOpType.max, op1=mybir.AluOpType.add)
                    ps2 = gps.tile([128, 128], fp32, tag="g")
                    nc.tensor.transpose(out=ps2[:], in_=hs[:], identity=ident[:])
                    hT = tl.tile([128, 128], fp32, tag="hT")
                    nc.vector.tensor_copy(out=hT[:], in_=ps2[:])
                    g2p = gps.tile([128, 128], fp32, tag="g")
                    nc.tensor.matmul(g2p[:, 0:68], lhsT=hT[:], rhs=g2eW[:],
                                     start=True, stop=True)
                    g2s = tl.tile([128, 68], b16, tag="g2s")
                    nc.vector.tensor_copy(out=g2s[:], in_=g2p[:, 0:68])
                    nc.sync.dma_start(out=o_g2[128 * t:128 * (t + 1), :], in_=g2s[:])
                    nc.scalar.activation(out=a2d_all[:, t:t + 1],
                                         in_=g2p[:, 65:66],
                                         func=mybir.ActivationFunctionType.Identity,
                                         bias=adj_t[:])
            cs = per.tile([128, cfg.TILES * 64], b16, tag="spreb")
            nc.vector.tensor_copy(out=cs[:], in_=spre[:])
            nc.sync.dma_start(out=o_spre[:], in_=cs[:])
            nc.sync.dma_start(out=o_a2d[:], in_=a2d_all[:])
    nc.finalize()
    return nc


def build_exec2(cfg, con, chunks, offs):
    """Program 2: GAT2 aggregation from host-gathered g2e grids + classifier."""
    import concourse.bacc as bacc
    import concourse.mybir as mybir
    import concourse.tile as tile
    from concourse.masks import make_identity

    fp32 = mybir.dt.float32
    b16 = mybir.dt.bfloat16
    S = int(offs[-1])

    nc = bacc.Bacc(None, target_bir_lowering=False, debug=False)
    ge = nc.dram_tensor("ge", [128, S * 72], b16, kind="ExternalInput")
    spre_i = nc.dram_tensor("spre_i", [128, cfg.TILES * 64], b16, kind="ExternalInput")
    a2d_i = nc.dram_tensor("a2d_i", [128, cfg.TILES], fp32, kind="ExternalInput")
    c_cw1 = nc.dram_tensor("c_cw1", [128, 64], fp32, kind="ExternalInput")
    c_cb1 = nc.dram_tensor("c_cb1", [128, 64], fp32, kind="ExternalInput")
    c_cw2 = nc.dram_tensor("c_cw2", [128, 64], fp32, kind="ExternalInput")
    o_p = nc.dram_tensor("o_p", [128, cfg.TILES], fp32, kind="ExternalOutput")
    cb2v = getattr(build_exec2, "_cb2", 0.0)

    with tile.TileContext(nc) as tc:
        with (
            tc.tile_pool(name="const", bufs=1) as cp,
            tc.tile_pool(name="per", bufs=1) as per,
            tc.tile_pool(name="geb", bufs=2) as gep,
            tc.tile_pool(name="ed", bufs=2) as ed,
            tc.tile_pool(name="tl", bufs=3) as tl,
            tc.tile_pool(name="gps", bufs=2, space="PSUM") as gps,
        ):
            ident = cp.tile([128, 128], fp32)
            make_identity(nc, ident[:])
            cw1 = cp.tile([128, 64], fp32)
            nc.sync.dma_start(out=cw1[:], in_=c_cw1[:])
            cb1 = cp.tile([128, 64], fp32)
            nc.sync.dma_start(out=cb1[:], in_=c_cb1[:])
            cw2 = cp.tile([128, 64], fp32)
            nc.sync.dma_start(out=cw2[:], in_=c_cw2[:])
            cb2_t = cp.tile([128, 1], fp32)
            nc.vector.memset(cb2_t[:], float(cb2v))
            spre = per.tile([128, cfg.TILES * 64], b16)
            nc.sync.dma_start(out=spre[:], in_=spre_i[:])
            a2d = per.tile([128, cfg.TILES], fp32)
            nc.sync.dma_start(out=a2d[:], in_=a2d_i[:])
            prob = per.tile([128, cfg.TILES], fp32)

            for ci, (t0, nt, dc) in enumerate(chunks):
                W = nt * dc
                geb = gep.tile([128, 128 * 72], b16, tag="geb")
                nc.sync.dma_start(out=geb[:, 0:W * 72],
                                  in_=ge[:, int(offs[ci]) * 72:(int(offs[ci]) + W) * 72])
                es2 = ed.tile([128, 128], b16, tag="es2")
                nc.vector.tensor_tensor(
                    out=es2[:, 0:W],
                    in0=_mk(geb.tensor, 64, [geb[:].ap[0], [72 * dc, nt], [72, dc]]),
                    in1=_mk(a2d.tensor, t0, [a2d[:].ap[0], [1, nt], [0, dc]]),
                    op=mybir.AluOpType.add)
                lr2 = ed.tile([128, 128], b16, tag="lr2")
                nc.vector.scalar_tensor_tensor(
                    out=lr2[:, 0:W], in0=es2[:, 0:W], scalar=0.2, in1=es2[:, 0:W],
                    op0=mybir.AluOpType.mult, op1=mybir.AluOpType.max)
                w2 = ed.tile([128, 128], b16, tag="w2")
                nc.scalar.activation(out=w2[:, 0:W], in_=lr2[:, 0:W],
                                     func=mybir.ActivationFunctionType.Exp)
                den2 = ed.tile([128, 16], fp32, tag="den2")
                nc.vector.tensor_reduce(
                    out=den2[:, 0:nt],
                    in_=_mk(w2.tensor, 0, [w2[:].ap[0], [dc, nt], [1, dc]]),
                    axis=mybir.AxisListType.X, op=mybir.AluOpType.add)
                rec2 = ed.tile([128, 16], fp32, tag="rec2")
                nc.vector.reciprocal(out=rec2[:, 0:nt], in_=den2[:, 0:nt])
                w2n = ed.tile([128, 128], b16, tag="w2n")
                nc.vector.tensor_tensor(
                    out=w2n[:, 0:W], in0=w2[:, 0:W],
                    in1=_mk(rec2.tensor, 0, [rec2[:].ap[0], [1, nt], [0, dc]]),
                    op=mybir.AluOpType.mult)
                tmp2 = ed.tile([128, 128 * 64], b16, tag="tmp2")
                nc.vector.tensor_tensor(
                    out=_mk(tmp2.tensor, 0,
                            [tmp2[:].ap[0], [64 * dc, nt], [64, dc], [1, 64]]),
                    in0=_mk(geb.tensor, 0,
                            [geb[:].ap[0], [72 * dc, nt], [72, dc], [1, 64]]),
                    in1=_mk(w2n.tensor, 0,
                            [w2n[:].ap[0], [dc, nt], [1, dc], [0, 64]]),
                    op=mybir.AluOpType.mult)
                out2 = ed.tile([128, 128 // 8 * 64], fp32, tag="out2")
                nc.vector.tensor_reduce(
                    out=out2[:, 0:nt * 64],
                    in_=_mk(tmp2.tensor, 0,
                            [tmp2[:].ap[0], [64 * dc, nt], [1, 64], [64, dc]]),
                    axis=mybir.AxisListType.X, op=mybir.AluOpType.add)
                for ti in range(nt):
                    t = t0 + ti
                    sp32 = tl.tile([128, 64], fp32, tag="sp32")
                    nc.vector.tensor_copy(out=sp32[:],
                                          in_=spre[:, 64 * t:64 * (t + 1)])
                    ps1 = gps.tile([128, 128], fp32, tag="g")
                    nc.tensor.transpose(out=ps1[0:64, :], in_=sp32[:],
                                        identity=ident[:])
                    stag = tl.tile([128, 128], fp32, tag="stag")
                    nc.vector.tensor_copy(out=stag[0:64, :], in_=ps1[0:64, :])
                    ps2 = gps.tile([128, 128], fp32, tag="g")
                    nc.tensor.transpose(out=ps2[0:64, :],
                                        in_=out2[:, 64 * ti:64 * (ti + 1)],
                                        identity=ident[:])
                    nc.vector.tensor_copy(out=stag[64:128, :], in_=ps2[0:64, :])
                    z1 = gps.tile([128, 64], fp32, tag="g")
                    nc.tensor.matmul(z1[:], lhsT=stag[:], rhs=cw1[:],
                                     start=True, stop=True)
                    y = tl.tile([128, 64], fp32, tag="y")
                    nc.vector.tensor_add(out=y[:], in0=z1[:], in1=cb1[:])
                    nc.vector.tensor_scalar_max(out=y[:], in0=y[:], scalar1=0.0)
                    zt = tl.tile([128, 64], fp32, tag="zt")
                    nc.vector.tensor_tensor(out=zt[:], in0=y[:], in1=cw2[:],
                                            op=mybir.AluOpType.mult)
                    zz = tl.tile([128, 1], fp32, tag="zz")
                    nc.vector.tensor_reduce(out=zz[:], in_=zt[:],
                                            axis=mybir.AxisListType.X,
                                            op=mybir.AluOpType.add)
                    nc.scalar.activation(out=prob[:, t:t + 1], in_=zz[:],
                                         func=mybir.ActivationFunctionType.Sigmoid,
                                         bias=cb2_t[:])
            nc.sync.dma_start(out=o_p[:], in_=prob[:])
    nc.finalize()
    return nc


# ======================================================================
# top level
# ======================================================================
def _run(nc, in_maps, ncore):
    from concourse.bass_utils import run_bass_kernel_spmd
    return run_bass_kernel_spmd(nc, in_maps, core_ids=list(range(ncore))).results


def kernel(temporal_data, x, edge_index, tW1, tb1, tW2, tb2,
           gW1, ga1_src, ga1_dst, gb1, gW2, ga2_src, ga2_dst, gb2,
           cW1, cb1, cW2, cb2, _cfg=None, _runner=None):
    cfg = _cfg or CFG
    x = np.asarray(x, np.float32)
    td = np.asarray(temporal_data, np.float32)
    w = dict(tW1=np.asarray(tW1, np.float32), tb1=np.asarray(tb1, np.float32),
             tW2=np.asarray(tW2, np.float32), tb2=np.asarray(tb2, np.float32),
             gW1=np.asarray(gW1, np.float32), ga1_src=np.asarray(ga1_src, np.float32),
             ga1_dst=np.asarray(ga1_dst, np.float32), gb1=np.asarray(gb1, np.float32),
             gW2=np.asarray(gW2, np.float32), ga2_src=np.asarray(ga2_src, np.float32),
             ga2_dst=np.asarray(ga2_dst, np.float32), gb2=np.asarray(gb2, np.float32),
             cW1=np.asarray(cW1, np.float32), cb1=np.asarray(cb1, np.float32),
             cW2=np.asarray(cW2, np.float32), cb2=np.asarray(cb2, np.float32))

    percore, invs, D = _prep_graph(cfg, edge_index)
    chunks, offs = _chunk_sched(cfg, D)
    S = int(offs[-1])
    cch = cfg.cchunks()
    con, adj2, cb2v = _prep_weights(cfg, w)
    ag = _a1d_groups(cfg)

    # A1s with bias-marker row: [11, 4] -> broadcast (h, k) layout [128, 44]
    gW1f = w["gW1"].astype(np.float32)
    A1s = (gW1f.reshape(cfg.F, cfg.H1, cfg.C1) * w["ga1_src"][None]).sum(-1)
    a1sp = np.zeros((11, 4), np.float32)
    a1sp[:cfg.F] = A1s
    a1sp[cfg.F] = 1.0
    a1spbc = np.tile(a1sp.T.reshape(1, 44), (128, 1))

    in1 = []
    flats = []
    for c in range(cfg.NCORE):
        flat, _ = _edge_layout(cfg, percore[c], D, chunks, offs)
        flats.append(flat)
        xg = _xe_grid(cfg, x, percore[c], flat, S)
        xg = _xe_fix_dummies(cfg, xg, percore[c], chunks, offs, x)
        in1.append({
            "td": _pack_td(cfg, td, percore[c][3], c).reshape(cfg.TILES * 128, 640),
            "xp": _pack_perm_x(cfg, x, percore[c][3], c),
            "xe": xg,
            "c_mm1f": con["rhs_mm1f"].astype(bf16),
            "c_mm1p": con["rhs_mm1p"].astype(bf16),
            "c_a1d": con["rhs_a1d"].astype(bf16),
            "c_a1sp": a1spbc.astype(bf16),
            "c_g1": con["rhs_g1"].astype(np.float32),
            "c_gb1": con["gb1bc"].astype(np.float32),
            "c_g2e": con["gw2ext"].astype(np.float32),
        })
        if ag[-1] != 8:
            in1[-1]["c_a1dp"] = con["rhs_a1d_p"].astype(bf16)

    nc1 = build_exec1(cfg, con, adj2, chunks, offs)
    runner = _runner or _run
    res1 = runner(nc1, in1, cfg.NCORE)

    g2cat_parts = []
    for (r0, r1, base) in cch:
        for c in range(cfg.NCORE):
            g2cat_parts.append(np.asarray(res1[c]["o_g2"])[r0:r1])
    g2cat = np.concatenate(g2cat_parts, 0)

    in2 = []
    for c in range(cfg.NCORE):
        gg, _ = _g2e_grid(cfg, g2cat, percore[c], invs, cch, flats[c], S)
        gg = _g2e_fix_dummies(cfg, gg, percore[c], chunks, offs, g2cat, invs, cch)
        in2.append({
            "ge": gg,
            "spre_i": np.asarray(res1[c]["o_spre"]),
            "a2d_i": np.asarray(res1[c]["o_a2d"]),
            "c_cw1": con["cw1f"].astype(np.float32),
            "c_cb1": con["cb1bc"].astype(np.float32),
            "c_cw2": con["cw2bc"].astype(np.float32),
        })
    build_exec2._cb2 = cb2v
    nc2 = build_exec2(cfg, con, chunks, offs)
    res2 = runner(nc2, in2, cfg.NCORE)

    out = np.zeros((cfg.N, 1), np.float32)
    for c in range(cfg.NCORE):
        p = np.asarray(res2[c]["o_p"])          # [128, TILES] (lane, tile)
        pl = p.T.reshape(cfg.LP)                # perm position -> prob
        out[c * cfg.L:(c + 1) * cfg.L, 0] = pl[invs[c]]
    return out



# revision 4
# speedup vs baseline: 5.0221x; 5.0221x over previous
"""Trainium2 Bass kernel for nn_FraudDetectionModel (temporal encoder + 2-layer
GAT + classifier). Self-contained: hardcodes shapes, shards across 8 cores.

Single device program: temporal encoder + GAT1 per destination-sharded edge
chunks; GAT1 node outputs written to a local HBM pair-row table, AllGathered
across the 8 cores, and per-edge source rows fetched on-device with GPSIMD
dma_gather (int16 A/B row-range split + flag-mask merge); GAT2 segment softmax
and the classifier finish in the same program. Only the [N,1] probabilities
leave the device.
"""
import numpy as np
import ml_dtypes

bf16 = ml_dtypes.bfloat16
NEG = -1.0e9
SPL = 32768              # int16 gather row-range split


class _Cfg:
    def __init__(self, n=100000, e=1600000, ncore=8):
        self.N, self.E, self.NCORE = n, e, ncore
        self.T, self.F = 50, 10
        self.H1, self.C1, self.LAT = 4, 32, 64
        self.L = n // ncore
        self.TILES = (self.L + 127) // 128
        self.LP = self.TILES * 128
        self.PAIRS = self.LP // 2                 # local pair rows
        self.NR2 = ncore * self.PAIRS             # gathered pair rows
        # temporal K-tiling
        self.TQ = [11, 11, 11, 11, 6]
        self.TQOFF = [0, 11, 22, 33, 44]
        self.KQ = [121, 121, 121, 121, 66]
        self.QW = [704, 704, 704, 704, 384]       # matmul cols per q (64*tq)


CFG = _Cfg()


# ======================================================================
# host prep: graph
# ======================================================================
def _prep_graph(cfg, edge_index):
    src = edge_index[0].astype(np.int64)
    dst = edge_index[1].astype(np.int64)
    loops = np.arange(cfg.N, dtype=np.int64)
    src = np.concatenate([src, loops])
    dst = np.concatenate([dst, loops])
    core = dst // cfg.L
    percore, invs = [], []
    for c in range(cfg.NCORE):
        m = core == c
        es, ed = src[m], dst[m] - c * cfg.L
        deg = np.bincount(ed, minlength=cfg.L)
        perm = np.argsort(deg, kind="stable")
        inv = np.empty(cfg.L, np.int64)
        inv[perm] = np.arange(cfg.L)
        degp = np.zeros(cfg.LP, np.int64)
        degp[:cfg.L] = deg[perm]
        pos = inv[ed]
        order = np.argsort(pos, kind="stable")
        percore.append((es[order], pos[order], degp, perm))
        invs.append(inv)
    D = np.ones(cfg.TILES, np.int64)
    for es, pos, degp, perm in percore:
        D = np.maximum(D, degp.reshape(cfg.TILES, 128).max(1))
    return percore, invs, D


def _chunk_sched(cfg, D, cap_slots=64, max_tiles=6):
    cap_slots = max(cap_slots, int(D.max()))
    chunks, t = [], 0
    while t < cfg.TILES:
        nt = 1
        while (t + nt < cfg.TILES and nt < max_tiles
               and int(max(D[t:t + nt + 1])) * (nt + 1) <= cap_slots):
            nt += 1
        chunks.append((t, nt, int(max(D[t:t + nt]))))
        t += nt
    offs = np.concatenate([[0], np.cumsum([nt * dc for _, nt, dc in chunks])])
    return chunks, offs.astype(np.int64)


def _edge_layout(cfg, percore_c, chunks, offs):
    """Flat slot index (lane*S + col) per edge for one core."""
    es, pos, degp, perm = percore_c
    S = int(offs[-1])
    start = np.concatenate([[0], np.cumsum(degp)])[:-1]
    rr = np.arange(len(pos)) - start[pos]
    tile_of = pos // 128
    lane = pos % 128
    tile2col = np.zeros(cfg.TILES, np.int64)
    for ci, (t0, nt, dc) in enumerate(chunks):
        tile2col[t0:t0 + nt] = offs[ci] + np.arange(nt) * dc
    col = tile2col[tile_of] + rr
    return lane * S + col, S, tile2col


# ======================================================================
# host prep: per-edge grids
# ======================================================================
def _xe_grid(cfg, x, a1s, percore_c, flat, S, tile2col):
    """xe [128, S*16] bf16: [x(10), a1s_h0..h3, 0, 0]; pads: a1s cols = NEG.
    Dummy slot 0 of zero-degree lanes = this core's perm-slot-0 node."""
    es, pos, degp, perm = percore_c
    g = np.zeros((128 * S, 16), np.float32)
    g[:, cfg.F:cfg.F + cfg.H1] = NEG
    g[flat, :cfg.F] = x[es]
    g[flat, cfg.F:cfg.F + cfg.H1] = a1s[es]
    dummy = np.nonzero(degp == 0)[0]
    if len(dummy):
        lanes, tiles = dummy % 128, dummy // 128
        dflat = lanes * S + tile2col[tiles]
        n0 = perm[0]
        g[dflat, :cfg.F] = x[n0]
        g[dflat, cfg.F:cfg.F + cfg.H1] = a1s[n0]
    return g.reshape(128, S * 16).astype(bf16)


GMAX = 8                 # gather group width (<=1024 idxs per dma_gather)


def _wrap16_chunks(lin2d, chunks, offs):
    """lin2d [128, S] slot-major wrap per <=GMAX-col group -> [16, 8*S] int16."""
    S = lin2d.shape[1]
    out = np.zeros((16, 8 * S), np.int16)
    for ci, (t0, nt, dc) in enumerate(chunks):
        o0, o1 = int(offs[ci]), int(offs[ci + 1])
        for g0 in range(o0, o1, GMAX):
            g1 = min(g0 + GMAX, o1)
            lin = lin2d[:, g0:g1].T.reshape(-1)   # i = (col-g0)*128 + lane
            n = len(lin)
            w = np.zeros((16, n // 16), np.int16)
            w[np.arange(n) % 16, np.arange(n) // 16] = lin
            out[:, g0 * 8:g1 * 8] = w
    return out


def _gat2_planes(cfg, percore_c, invs, c, flat, S, tile2col):
    """Gather planes for GAT2: idxa/idxb [16, 8S] i16, flags [128, S] bf16."""
    es, pos, degp, perm = percore_c
    owner = es // cfg.L
    r = np.empty_like(es)
    for oc in range(cfg.NCORE):
        m = owner == oc
        if m.any():
            r[m] = invs[oc][es[m] - oc * cfg.L]
    grow = owner * cfg.LP + r
    pair = grow // 2
    half = grow % 2
    isb = pair >= SPL

    idxa = np.zeros((128 * S,), np.int64)
    idxb = np.zeros((128 * S,), np.int64)
    flags = np.full((128 * S,), 9.0, np.float32)
    idxa[flat] = np.where(isb, 0, pair)
    idxb[flat] = np.where(isb, pair - SPL, 0)
    flags[flat] = 2 * isb.astype(np.int64) + half

    dummy = np.nonzero(degp == 0)[0]
    if len(dummy):
        lanes, tiles = dummy % 128, dummy // 128
        dflat = lanes * S + tile2col[tiles]
        dgrow = c * cfg.LP + 0                    # own perm-slot-0 node
        dpair, dhalf = dgrow // 2, dgrow % 2
        if dpair >= SPL:
            idxa[dflat] = 0
            idxb[dflat] = dpair - SPL
            flags[dflat] = 2 + dhalf
        else:
            idxa[dflat] = dpair
            idxb[dflat] = 0
            flags[dflat] = dhalf

    idxa = idxa.reshape(128, S)
    idxb = idxb.reshape(128, S)
    return (_wrap16_chunks(idxa, *_cho(cfg)), _wrap16_chunks(idxb, *_cho(cfg)),
            flags.reshape(128, S).astype(bf16))


_CHO = None


def _cho(cfg):
    return _CHO


# ======================================================================
# host prep: packing
# ======================================================================
def _pack_td(cfg, td, perm, c):
    tdp = np.zeros((cfg.LP, cfg.T, cfg.F), np.float32)
    tdp[:cfg.L] = td[c * cfg.L + perm]
    out = np.zeros((cfg.TILES, 128, 640), np.float32)
    nodes = tdp.reshape(cfg.TILES, 128, cfg.T, cfg.F)
    for q in range(5):
        tq, kq = cfg.TQ[q], cfg.KQ[q]
        blk = nodes[:, :, cfg.TQOFF[q]:cfg.TQOFF[q] + tq, :]
        r = np.zeros((cfg.TILES, tq, 11, 128), np.float32)
        r[:, :, :cfg.F] = blk.transpose(0, 2, 3, 1)
        r[:, :, cfg.F] = 1.0
        out[:, :kq, 128 * q:128 * q + 128] = r.reshape(cfg.TILES, kq, 128)
    return np.ascontiguousarray(out).astype(bf16)


def _block_diag(b, n):
    r, c = b.shape
    out = np.zeros((n * r, n * c), np.float32)
    for i in range(n):
        out[i * r:(i + 1) * r, i * c:(i + 1) * c] = b
    return out


def _prep_weights(cfg, w):
    F, H1, C1, LAT = cfg.F, cfg.H1, cfg.C1, cfg.LAT
    gW1 = w["gW1"].astype(np.float32)
    A1s = (gW1.reshape(F, H1, C1) * w["ga1_src"][None]).sum(-1)
    A1d = (gW1.reshape(F, H1, C1) * w["ga1_dst"][None]).sum(-1)
    gW2 = w["gW2"].astype(np.float32)
    A2s = (gW2.reshape(H1 * C1, 1, LAT) * w["ga2_src"][None]).sum(-1)[:, 0]
    A2d = (gW2.reshape(H1 * C1, 1, LAT) * w["ga2_dst"][None]).sum(-1)[:, 0]

    con = {}
    W1b = np.concatenate([w["tW1"], w["tb1"][None]], 0).astype(np.float32)
    con["rhs_mm1f"] = np.concatenate(
        [_block_diag(W1b, 11), np.tile(W1b, (11, 1))], 1)     # [121, 768]
    con["rhs_mm1p"] = np.concatenate(
        [_block_diag(W1b, 6), np.tile(W1b, (6, 1))], 1)       # [66, 448]

    rhs_g1 = np.zeros((4 * F, H1 * C1), np.float32)
    for h in range(H1):
        rhs_g1[h * F:(h + 1) * F, h * C1:(h + 1) * C1] = gW1[:, h * C1:(h + 1) * C1]
    con["rhs_g1"] = rhs_g1                                    # [40, 128] f32
    con["gb1bc"] = np.tile(w["gb1"][None], (128, 1))          # [128, 128] f32

    gw2ext = np.zeros((H1 * C1, 68), np.float32)
    gw2ext[:, :LAT] = gW2
    gw2ext[:, LAT] = A2s
    gw2ext[:, LAT + 1] = A2d
    con["gw2ext"] = gw2ext                                    # [128, 68] f32

    ones = np.ones(H1 * C1, np.float32)
    adj2 = float(-(ones @ A2s) - (ones @ A2d))
    g2shift = ones @ gW2
    cW1 = w["cW1"].astype(np.float32)
    con["cw1f"] = np.concatenate(
        [(w["tW2"] / 100.0) @ cW1[:LAT], cW1[LAT:]], 0)       # [128, 64] f32
    cb1p = (w["tb2"] @ cW1[:LAT]) + ((w["gb2"] - g2shift) @ cW1[LAT:]) + w["cb1"]
    con["cb1bc"] = np.tile(cb1p[None], (128, 1))              # [128, 64] f32
    con["cw2bc"] = np.tile(w["cW2"][:, 0][None], (128, 1))    # [128, 64] f32
    return con, adj2, float(w["cb2"][0]), A1s, A1d


# ======================================================================
# device builder: single program
# ======================================================================
def _mk(ap_tensor, offset, dims):
    from concourse.bass import AP
    return AP(ap_tensor, int(offset), [list(d) for d in dims])


def build_exec(cfg, chunks, offs, adj2, cb2v):
    import concourse.bacc as bacc
    import concourse.mybir as mybir
    import concourse.tile as tile
    from concourse.masks import make_identity

    fp32 = mybir.dt.float32
    b16 = mybir.dt.bfloat16
    i16 = mybir.dt.int16
    F, H1, TQ, KQ, QW = cfg.F, cfg.H1, cfg.TQ, cfg.KQ, cfg.QW
    S = int(offs[-1])
    WMAX = max(nt * dc for _, nt, dc in chunks)
    NTMAX = max(nt for _, nt, dc in chunks)

    nc = bacc.Bacc(None, target_bir_lowering=False, debug=False)
    td = nc.dram_tensor("td", [cfg.TILES * 128, 640], b16, kind="ExternalInput")
    xe = nc.dram_tensor("xe", [128, S * 16], b16, kind="ExternalInput")
    a1d_i = nc.dram_tensor("a1d_i", [128, cfg.TILES * 4], b16, kind="ExternalInput")
    idxa = nc.dram_tensor("idxa", [16, 8 * S], i16, kind="ExternalInput")
    idxb = nc.dram_tensor("idxb", [16, 8 * S], i16, kind="ExternalInput")
    flg = nc.dram_tensor("flg", [128, S], b16, kind="ExternalInput")
    c_mm1f = nc.dram_tensor("c_mm1f", [121, 768], b16, kind="ExternalInput")
    c_mm1p = nc.dram_tensor("c_mm1p", [66, 448], b16, kind="ExternalInput")
    c_g1 = nc.dram_tensor("c_g1", [40, 128], fp32, kind="ExternalInput")
    c_gb1 = nc.dram_tensor("c_gb1", [128, 128], fp32, kind="ExternalInput")
    c_g2e = nc.dram_tensor("c_g2e", [128, 68], fp32, kind="ExternalInput")
    c_cw1 = nc.dram_tensor("c_cw1", [128, 64], fp32, kind="ExternalInput")
    c_cb1 = nc.dram_tensor("c_cb1", [128, 64], fp32, kind="ExternalInput")
    c_cw2 = nc.dram_tensor("c_cw2", [128, 64], fp32, kind="ExternalInput")
    o_p = nc.dram_tensor("o_p", [128, cfg.TILES], fp32, kind="ExternalOutput")

    with tile.TileContext(nc) as tc:
        with (
            tc.tile_pool(name="dram", bufs=1, space="DRAM") as dram,
            tc.tile_pool(name="const", bufs=1) as cp,
            tc.tile_pool(name="per", bufs=1) as per,
            tc.tile_pool(name="tds", bufs=3) as tds,
            tc.tile_pool(name="qps", bufs=2, space="PSUM") as qps,
            tc.tile_pool(name="lps", bufs=2, space="PSUM") as lps,
            tc.tile_pool(name="gps", bufs=2, space="PSUM") as gps,
            tc.tile_pool(name="ev", bufs=2) as ev,
            tc.tile_pool(name="xep", bufs=2) as xep,
            tc.tile_pool(name="ed", bufs=2) as ed,
            tc.tile_pool(name="tl", bufs=3) as tl,
            tc.tile_pool(name="gth", bufs=1) as gth,
            tc.tile_pool(name="idxp", bufs=2) as idxp,
            tc.tile_pool(name="mrg", bufs=2) as mrg,
        ):
            g2loc = dram.tile([cfg.PAIRS, 256], b16)
            g2all = dram.tile([cfg.NCORE * cfg.PAIRS, 256], b16)

            ident = cp.tile([128, 128], fp32)
            make_identity(nc, ident[:])
            mm1f = cp.tile([121, 768], b16)
            nc.sync.dma_start(out=mm1f[:], in_=c_mm1f[:])
            mm1p = cp.tile([66, 448], b16)
            nc.sync.dma_start(out=mm1p[:], in_=c_mm1p[:])
            g1W = cp.tile([40, 128], fp32)
            nc.sync.dma_start(out=g1W[:], in_=c_g1[:])
            gb1bc = cp.tile([128, 128], fp32)
            nc.sync.dma_start(out=gb1bc[:], in_=c_gb1[:])
            g2eW = cp.tile([128, 68], fp32)
            nc.sync.dma_start(out=g2eW[:], in_=c_g2e[:])
            cw1 = cp.tile([128, 64], fp32)
            nc.sync.dma_start(out=cw1[:], in_=c_cw1[:])
            cb1 = cp.tile([128, 64], fp32)
            nc.sync.dma_start(out=cb1[:], in_=c_cb1[:])
            cw2 = cp.tile([128, 64], fp32)
            nc.sync.dma_start(out=cw2[:], in_=c_cw2[:])
            adj_t = cp.tile([128, 1], fp32)
            nc.vector.memset(adj_t[:], float(adj2))
            cb2_t = cp.tile([128, 1], fp32)
            nc.vector.memset(cb2_t[:], float(cb2v))
            a1d_all = per.tile([128, cfg.TILES * 4], b16)
            nc.sync.dma_start(out=a1d_all[:], in_=a1d_i[:])

            spre = per.tile([128, cfg.TILES * 64], b16)
            a2d_all = per.tile([128, cfg.TILES], fp32)
            prob = per.tile([128, cfg.TILES], fp32)

            # ---- GAT1 per chunk -> g2loc pair rows ----
            for ci, (t0, nt, dc) in enumerate(chunks):
                W = nt * dc
                xeb = xep.tile([128, WMAX * 16], b16, tag="xeb")
                nc.sync.dma_start(out=xeb[:, 0:W * 16],
                                  in_=xe[:, int(offs[ci]) * 16:(int(offs[ci]) + W) * 16])
                # esum layout (h, t, j): xe a1s col + a1d_dst
                esum = ed.tile([128, WMAX * 4], b16, tag="esum")
                for h in range(4):
                    nc.vector.tensor_tensor(
                        out=esum[:, h * W:(h + 1) * W],
                        in0=_mk(xeb.tensor, F + h,
                                [xeb[:].ap[0], [16 * dc, nt], [16, dc]]),
                        in1=_mk(a1d_all.tensor, 4 * t0 + h,
                                [a1d_all[:].ap[0], [4, nt], [0, dc]]),
                        op=mybir.AluOpType.add)
                lr = ed.tile([128, WMAX * 4], b16, tag="lr")
                nc.vector.scalar_tensor_tensor(
                    out=lr[:, 0:W * 4], in0=esum[:, 0:W * 4], scalar=0.2,
                    in1=esum[:, 0:W * 4],
                    op0=mybir.AluOpType.mult, op1=mybir.AluOpType.max)
                wv = ed.tile([128, WMAX * 4], b16, tag="wv")
                nc.scalar.activation(out=wv[:, 0:W * 4], in_=lr[:, 0:W * 4],
                                     func=mybir.ActivationFunctionType.Exp)
                den = ed.tile([128, NTMAX * 4], fp32, tag="den")
                nc.vector.tensor_reduce(
                    out=den[:, 0:nt * 4],
                    in_=_mk(wv.tensor, 0, [wv[:].ap[0], [dc, 4 * nt], [1, dc]]),
                    axis=mybir.AxisListType.X, op=mybir.AluOpType.add)
                rec = ed.tile([128, NTMAX * 4], fp32, tag="rec")
                nc.vector.reciprocal(out=rec[:, 0:nt * 4], in_=den[:, 0:nt * 4])
                tmpx = ed.tile([128, WMAX * 10], b16, tag="tmpx")
                xaggr = ed.tile([128, NTMAX * 40], fp32, tag="xaggr")
                for h in range(4):
                    nc.vector.tensor_tensor(
                        out=tmpx[:, 0:W * 10],
                        in0=_mk(xeb.tensor, 0,
                                [xeb[:].ap[0], [16 * dc, nt], [16, dc], [1, 10]]),
                        in1=_mk(wv.tensor, h * W,
                                [wv[:].ap[0], [dc, nt], [1, dc], [0, 10]]),
                        op=mybir.AluOpType.mult)
                    nc.vector.tensor_reduce(
                        out=_mk(xaggr.tensor, 10 * h,
                                [xaggr[:].ap[0], [40, nt], [1, 10]]),
                        in_=_mk(tmpx.tensor, 0,
                                [tmpx[:].ap[0], [10 * dc, nt], [1, 10], [10, dc]]),
                        axis=mybir.AxisListType.X, op=mybir.AluOpType.add)
                xagg = ed.tile([128, NTMAX * 40], fp32, tag="xagg")
                nc.vector.tensor_tensor(
                    out=xagg[:, 0:nt * 40],
                    in0=xaggr[:, 0:nt * 40],
                    in1=_mk(rec.tensor, 0,
                            [rec[:].ap[0], [1, nt], [nt, 4], [0, 10]]),
                    op=mybir.AluOpType.mult)
                for ti in range(nt):
                    t = t0 + ti
                    ps1 = gps.tile([128, 128], fp32, tag="g")
                    nc.tensor.transpose(out=ps1[0:40, :],
                                        in_=xagg[:, 40 * ti:40 * (ti + 1)],
                                        identity=ident[:])
                    stag = tl.tile([40, 128], fp32, tag="stag1")
                    nc.vector.tensor_copy(out=stag[:], in_=ps1[0:40, :])
                    out1 = gps.tile([128, 128], fp32, tag="g")
                    nc.tensor.matmul(out1[:], lhsT=stag[:], rhs=g1W[:],
                                     start=True, stop=True)
                    y1 = tl.tile([128, 128], fp32, tag="y1")
                    nc.vector.tensor_add(out=y1[:], in0=out1[:], in1=gb1bc[:])
                    mn = tl.tile([128, 128], fp32, tag="mn")
                    nc.vector.tensor_scalar_min(out=mn[:], in0=y1[:], scalar1=0.0)
                    ex = tl.tile([128, 128], fp32, tag="ex")
                    nc.scalar.activation(out=ex[:], in_=mn[:],
                                         func=mybir.ActivationFunctionType.Exp)
                    hs = tl.tile([128, 128], fp32, tag="hs")
                    nc.vector.scalar_tensor_tensor(
                        out=hs[:], in0=y1[:], scalar=0.0, in1=ex[:],
                        op0=mybir.AluOpType.max, op1=mybir.AluOpType.add)
                    ps2 = gps.tile([128, 128], fp32, tag="g")
                    nc.tensor.transpose(out=ps2[:], in_=hs[:], identity=ident[:])
                    hT = tl.tile([128, 128], fp32, tag="hT")
                    nc.vector.tensor_copy(out=hT[:], in_=ps2[:])
                    g2p = gps.tile([128, 128], fp32, tag="g")
                    nc.tensor.matmul(g2p[:, 0:68], lhsT=hT[:], rhs=g2eW[:],
                                     start=True, stop=True)
                    g2s = tl.tile([128, 68], b16, tag="g2s")
                    nc.vector.tensor_copy(out=g2s[:], in_=g2p[:, 0:68])
                    # pair-row write: row = 64t + p//2, half = p%2
                    nc.sync.dma_start(
                        out=_mk(g2loc.tensor, t * 64 * 256,
                                [[256, 64], [68, 2], [1, 68]]),
                        in_=g2s[:])
                    nc.scalar.activation(out=a2d_all[:, t:t + 1],
                                         in_=g2p[:, 65:66],
                                         func=mybir.ActivationFunctionType.Identity,
                                         bias=adj_t[:])

            # ---- halo exchange ----
            nc.gpsimd.collective_compute(
                "AllGather", mybir.AluOpType.bypass,
                replica_groups=[list(range(cfg.NCORE))],
                ins=[g2loc[:]], outs=[g2all[:]])

            # ---- temporal encoder ----
            for t in range(cfg.TILES):
                st = tds.tile([128, 640], b16)
                nc.sync.dma_start(out=st[:], in_=td[128 * t:128 * (t + 1), :])
                lin = lps.tile([128, 64], fp32)
                sacc = tl.tile([128, 64], fp32, tag="sacc")
                for q in range(5):
                    kq, w = KQ[q], QW[q]
                    rhs = mm1f if q < 4 else mm1p
                    ps = qps.tile([128, 704], fp32, tag="qtile")
                    lhsT = st[0:kq, 128 * q:128 * (q + 1)]
                    for c0 in range(0, w, 512):
                        c1 = min(c0 + 512, w)
                        nc.tensor.matmul(ps[:, c0:c1], lhsT=lhsT,
                                         rhs=rhs[0:kq, c0:c1], start=True, stop=True)
                    nc.tensor.matmul(lin[:], lhsT=lhsT,
                                     rhs=rhs[0:kq, QW[q]:QW[q] + 64],
                                     start=(q == 0), stop=(q == 4))
                    tq = TQ[q]
                    if q < 2:
                        red = sacc if q == 0 else tl.tile([128, 64], fp32, tag="tred")
                        nc.vector.tensor_reduce(
                            out=red[:],
                            in_=_mk(ps.tensor, 0, [ps[:].ap[0], [1, 64], [64, tq]]),
                            axis=mybir.AxisListType.X, op=mybir.AluOpType.add,
                            apply_absolute_value=True)
                    else:
                        stg = ev.tile([128, 704], b16, tag="evs")
                        nc.scalar.activation(out=stg[:, 0:w], in_=ps[:, 0:w],
                                             func=mybir.ActivationFunctionType.Abs)
                        red = tl.tile([128, 64], fp32, tag="tred")
                        nc.vector.tensor_reduce(
                            out=red[:],
                            in_=_mk(stg.tensor, 0, [stg[:].ap[0], [1, 64], [64, tq]]),
                            axis=mybir.AxisListType.X, op=mybir.AluOpType.add)
                    if q > 0:
                        nc.vector.tensor_add(out=sacc[:], in0=sacc[:], in1=red[:])
                nc.vector.tensor_add(out=spre[:, 64 * t:64 * (t + 1)],
                                     in0=sacc[:], in1=lin[:])

            # ---- GAT2 per chunk (gather + segment softmax) + classifier ----
            for ci, (t0, nt, dc) in enumerate(chunks):
                W = nt * dc
                o0 = int(offs[ci])
                ia = idxp.tile([128, WMAX * 8], i16, tag="ia")
                nc.sync.dma_start(
                    out=ia[:, 0:W * 8],
                    in_=_mk(idxa, o0 * 8, [[0, 8], [8 * S, 16], [1, W * 8]]))
                ib = idxp.tile([128, WMAX * 8], i16, tag="ib")
                nc.sync.dma_start(
                    out=ib[:, 0:W * 8],
                    in_=_mk(idxb, o0 * 8, [[0, 8], [8 * S, 16], [1, W * 8]]))
                fl = idxp.tile([128, WMAX], b16, tag="fl")
                nc.sync.dma_start(out=fl[:, 0:W],
                                  in_=_mk(flg, o0, [[S, 128], [1, W]]))
                gA = gth.tile([128, WMAX * 256], b16, tag="gA")
                gB = gth.tile([128, WMAX * 256], b16, tag="gB")
                for r0 in range(0, W, GMAX):
                    r1 = min(r0 + GMAX, W)
                    gw = r1 - r0
                    nc.gpsimd.dma_gather(
                        _mk(gA.tensor, r0 * 256, [gA[:].ap[0], [256, gw], [1, 256]]),
                        _mk(g2all.tensor, 0, [[256, SPL], [1, 256]]),
                        ia[:, r0 * 8:r1 * 8], gw * 128, gw * 128, 256)
                    nc.gpsimd.dma_gather(
                        _mk(gB.tensor, r0 * 256, [gB[:].ap[0], [256, gw], [1, 256]]),
                        _mk(g2all.tensor, SPL * 256,
                            [[256, cfg.NR2 - SPL], [1, 256]]),
                        ib[:, r0 * 8:r1 * 8], gw * 128, gw * 128, 256)
                m = []
                for k in range(4):
                    mk_ = mrg.tile([128, WMAX], b16, tag=f"m{k}")
                    nc.vector.tensor_scalar(
                        out=mk_[:, 0:W], in0=fl[:, 0:W], scalar1=float(k),
                        scalar2=None, op0=mybir.AluOpType.is_equal)
                    m.append(mk_)
                vm = mrg.tile([128, WMAX], b16, tag="vm")
                nc.vector.tensor_scalar(
                    out=vm[:, 0:W], in0=fl[:, 0:W], scalar1=4.0,
                    scalar2=None, op0=mybir.AluOpType.is_lt)
                ge68 = mrg.tile([128, WMAX * 68], b16, tag="ge68")
                t1 = mrg.tile([128, WMAX * 68], b16, tag="t1")
                t2 = mrg.tile([128, WMAX * 68], b16, tag="t2")
                for gt, m0, m1, first in ((gA, m[0], m[1], True),
                                          (gB, m[2], m[3], False)):
                    nc.vector.tensor_tensor(
                        out=t1[:, 0:W * 68],
                        in0=_mk(gt.tensor, 0, [gt[:].ap[0], [256, W], [1, 68]]),
                        in1=_mk(m0.tensor, 0, [m0[:].ap[0], [1, W], [0, 68]]),
                        op=mybir.AluOpType.mult)
                    nc.vector.tensor_tensor(
                        out=t2[:, 0:W * 68],
                        in0=_mk(gt.tensor, 68, [gt[:].ap[0], [256, W], [1, 68]]),
                        in1=_mk(m1.tensor, 0, [m1[:].ap[0], [1, W], [0, 68]]),
                        op=mybir.AluOpType.mult)
                    if first:
                        nc.vector.tensor_add(out=ge68[:, 0:W * 68],
                                             in0=t1[:, 0:W * 68], in1=t2[:, 0:W * 68])
                    else:
                        nc.vector.tensor_add(out=t1[:, 0:W * 68],
                                             in0=t1[:, 0:W * 68], in1=t2[:, 0:W * 68])
                        nc.vector.tensor_add(out=ge68[:, 0:W * 68],
                                             in0=ge68[:, 0:W * 68], in1=t1[:, 0:W * 68])
                # segment softmax over incoming edges
                es2 = ed.tile([128, WMAX], b16, tag="es2")
                nc.vector.tensor_tensor(
                    out=es2[:, 0:W],
                    in0=_mk(ge68.tensor, 64, [ge68[:].ap[0], [68, W]]),
                    in1=_mk(a2d_all.tensor, t0, [a2d_all[:].ap[0], [1, nt], [0, dc]]),
                    op=mybir.AluOpType.add)
                lr2 = ed.tile([128, WMAX], b16, tag="lr2")
                nc.vector.scalar_tensor_tensor(
                    out=lr2[:, 0:W], in0=es2[:, 0:W], scalar=0.2, in1=es2[:, 0:W],
                    op0=mybir.AluOpType.mult, op1=mybir.AluOpType.max)
                w2 = ed.tile([128, WMAX], b16, tag="w2")
                nc.scalar.activation(out=w2[:, 0:W], in_=lr2[:, 0:W],
                                     func=mybir.ActivationFunctionType.Exp)
                w2v = ed.tile([128, WMAX], b16, tag="w2v")
                nc.vector.tensor_tensor(out=w2v[:, 0:W], in0=w2[:, 0:W],
                                        in1=vm[:, 0:W], op=mybir.AluOpType.mult)
                den2 = ed.tile([128, NTMAX], fp32, tag="den2")
                nc.vector.tensor_reduce(
                    out=den2[:, 0:nt],
                    in_=_mk(w2v.tensor, 0, [w2v[:].ap[0], [dc, nt], [1, dc]]),
                    axis=mybir.AxisListType.X, op=mybir.AluOpType.add)
                rec2 = ed.tile([128, NTMAX], fp32, tag="rec2")
                nc.vector.reciprocal(out=rec2[:, 0:nt], in_=den2[:, 0:nt])
                w2n = ed.tile([128, WMAX], b16, tag="w2n")
                nc.vector.tensor_tensor(
                    out=w2n[:, 0:W], in0=w2v[:, 0:W],
                    in1=_mk(rec2.tensor, 0, [rec2[:].ap[0], [1, nt], [0, dc]]),
                    op=mybir.AluOpType.mult)
                tmp2 = ed.tile([128, WMAX * 64], b16, tag="tmp2")
                nc.vector.tensor_tensor(
                    out=tmp2[:, 0:W * 64],
                    in0=_mk(ge68.tensor, 0, [ge68[:].ap[0], [68, W], [1, 64]]),
                    in1=_mk(w2n.tensor, 0, [w2n[:].ap[0], [1, W], [0, 64]]),
                    op=mybir.AluOpType.mult)
                out2 = ed.tile([128, NTMAX * 64], fp32, tag="out2")
                nc.vector.tensor_reduce(
                    out=out2[:, 0:nt * 64],
                    in_=_mk(tmp2.tensor, 0,
                            [tmp2[:].ap[0], [64 * dc, nt], [1, 64], [64, dc]]),
                    axis=mybir.AxisListType.X, op=mybir.AluOpType.add)
                for ti in range(nt):
                    t = t0 + ti
                    sp32 = tl.tile([128, 64], fp32, tag="sp32")
                    nc.vector.tensor_copy(out=sp32[:],
                                          in_=spre[:, 64 * t:64 * (t + 1)])
                    ps1 = gps.tile([128, 128], fp32, tag="g")
                    nc.tensor.transpose(out=ps1[0:64, :], in_=sp32[:],
                                        identity=ident[:])
                    stag = tl.tile([128, 128], fp32, tag="stag2")
                    nc.vector.tensor_copy(out=stag[0:64, :], in_=ps1[0:64, :])
                    ps2 = gps.tile([128, 128], fp32, tag="g")
                    nc.tensor.transpose(out=ps2[0:64, :],
                                        in_=out2[:, 64 * ti:64 * (ti + 1)],
                                        identity=ident[:])
                    nc.vector.tensor_copy(out=stag[64:128, :], in_=ps2[0:64, :])
                    z1 = gps.tile([128, 64], fp32, tag="g")
                    nc.tensor.matmul(z1[:], lhsT=stag[:], rhs=cw1[:],
                                     start=True, stop=True)
                    y = tl.tile([128, 64], fp32, tag="y")
                    nc.vector.tensor_add(out=y[:], in0=z1[:], in1=cb1[:])
                    nc.vector.tensor_scalar_max(out=y[:], in0=y[:], scalar1=0.0)
                    zt = tl.tile([128, 64], fp32, tag="zt")
                    nc.vector.tensor_tensor(out=zt[:], in0=y[:], in1=cw2[:],
                                            op=mybir.AluOpType.mult)
                    zz = tl.tile([128, 1], fp32, tag="zz")
                    nc.vector.tensor_reduce(out=zz[:], in_=zt[:],
                                            axis=mybir.AxisListType.X,
                                            op=mybir.AluOpType.add)
                    nc.scalar.activation(out=prob[:, t:t + 1], in_=zz[:],
                                         func=mybir.ActivationFunctionType.Sigmoid,
                                         bias=cb2_t[:])
            nc.sync.dma_start(out=o_p[:], in_=prob[:])
    nc.finalize()
    return nc


# ======================================================================
# top level
# ======================================================================
def _run(nc, in_maps, ncore):
    from concourse.bass_utils import run_bass_kernel_spmd
    return run_bass_kernel_spmd(nc, in_maps, core_ids=list(range(ncore))).results


def kernel(temporal_data, x, edge_index, tW1, tb1, tW2, tb2,
           gW1, ga1_src, ga1_dst, gb1, gW2, ga2_src, ga2_dst, gb2,
           cW1, cb1, cW2, cb2, _cfg=None, _runner=None):
    global _CHO
    cfg = _cfg or CFG
    x = np.asarray(x, np.float32)
    td = np.asarray(temporal_data, np.float32)
    w = dict(tW1=np.asarray(tW1, np.float32), tb1=np.asarray(tb1, np.float32),
             tW2=np.asarray(tW2, np.float32), tb2=np.asarray(tb2, np.float32),
             gW1=np.asarray(gW1, np.float32), ga1_src=np.asarray(ga1_src, np.float32),
             ga1_dst=np.asarray(ga1_dst, np.float32), gb1=np.asarray(gb1, np.float32),
             gW2=np.asarray(gW2, np.float32), ga2_src=np.asarray(ga2_src, np.float32),
             ga2_dst=np.asarray(ga2_dst, np.float32), gb2=np.asarray(gb2, np.float32),
             cW1=np.asarray(cW1, np.float32), cb1=np.asarray(cb1, np.float32),
             cW2=np.asarray(cW2, np.float32), cb2=np.asarray(cb2, np.float32))

    percore, invs, D = _prep_graph(cfg, edge_index)
    chunks, offs = _chunk_sched(cfg, D)
    _CHO = (chunks, offs)
    con, adj2, cb2v, A1s, A1d = _prep_weights(cfg, w)
    a1s_all = x @ A1s                       # [N, 4]
    a1d_vals = x @ A1d                      # [N, 4]

    ins = []
    for c in range(cfg.NCORE):
        flat, S, tile2col = _edge_layout(cfg, percore[c], chunks, offs)
        perm = percore[c][3]
        a1dg = np.zeros((cfg.LP, 4), np.float32)
        a1dg[:cfg.L] = a1d_vals[c * cfg.L + perm]
        ia, ib, fl = _gat2_planes(cfg, percore[c], invs, c, flat, S, tile2col)
        ins.append({
            "td": _pack_td(cfg, td, perm, c).reshape(cfg.TILES * 128, 640),
            "xe": _xe_grid(cfg, x, a1s_all, percore[c], flat, S, tile2col),
            "a1d_i": a1dg.reshape(cfg.TILES, 128, 4).transpose(1, 0, 2)
                         .reshape(128, cfg.TILES * 4).astype(bf16),
            "idxa": ia, "idxb": ib, "flg": fl,
            "c_mm1f": con["rhs_mm1f"].astype(bf16),
            "c_mm1p": con["rhs_mm1p"].astype(bf16),
            "c_g1": con["rhs_g1"].astype(np.float32),
            "c_gb1": con["gb1bc"].astype(np.float32),
            "c_g2e": con["gw2ext"].astype(np.float32),
            "c_cw1": con["cw1f"].astype(np.float32),
            "c_cb1": con["cb1bc"].astype(np.float32),
            "c_cw2": con["cw2bc"].astype(np.float32),
        })

    nc = build_exec(cfg, chunks, offs, adj2, cb2v)
    runner = _runner or _run
    res = runner(nc, ins, cfg.NCORE)

    out = np.zeros((cfg.N, 1), np.float32)
    for c in range(cfg.NCORE):
        p = np.asarray(res[c]["o_p"])           # [128, TILES] (lane, tile)
        pl = p.T.reshape(cfg.LP)                # perm position -> prob
        out[c * cfg.L:(c + 1) * cfg.L, 0] = pl[invs[c]]
    return out


# revision 22
# speedup vs baseline: 10.2101x; 2.0330x over previous
"""Trainium2 Bass kernel for nn_FraudDetectionModel (temporal encoder + 2-layer
GAT + classifier). Self-contained: hardcodes shapes, shards across 8 cores.

Single device program: temporal encoder + GAT1 per destination-sharded edge
chunks; GAT1 node outputs written to a local HBM pair-row table, AllGathered
across the 8 cores, and per-edge source rows fetched on-device with GPSIMD
dma_gather (int16 A/B row-range split + flag-mask merge); GAT2 segment softmax
and the classifier finish in the same program. Only the [N,1] probabilities
leave the device.
"""
import numpy as np
import ml_dtypes

bf16 = ml_dtypes.bfloat16
NEG = -1.0e9
SPL = 32768              # int16 gather row-range split


class _Cfg:
    def __init__(self, n=100000, e=1600000, ncore=8):
        self.N, self.E, self.NCORE = n, e, ncore
        self.T, self.F = 50, 10
        self.H1, self.C1, self.LAT = 4, 32, 64
        self.L = n // ncore
        self.TILES = (self.L + 127) // 128
        self.LP = self.TILES * 128
        self.PAIRS = self.LP // 2                 # local pair rows
        self.NR2 = ncore * self.PAIRS             # gathered pair rows
        # temporal K-tiling
        self.TQ = [11, 11, 11, 11, 6]
        self.TQOFF = [0, 11, 22, 33, 44]
        self.KQ = [121, 121, 121, 121, 66]
        self.QW = [704, 704, 704, 704, 384]       # matmul cols per q (64*tq)


CFG = _Cfg()


# ======================================================================
# host prep: graph
# ======================================================================
def _prep_graph(cfg, edge_index):
    src = edge_index[0].astype(np.int64)
    dst = edge_index[1].astype(np.int64)
    loops = np.arange(cfg.N, dtype=np.int64)
    src = np.concatenate([src, loops])
    dst = np.concatenate([dst, loops])
    core = dst // cfg.L
    percore, invs = [], []
    for c in range(cfg.NCORE):
        m = core == c
        es, ed = src[m], dst[m] - c * cfg.L
        deg = np.bincount(ed, minlength=cfg.L)
        perm = np.argsort(deg, kind="stable")
        inv = np.empty(cfg.L, np.int64)
        inv[perm] = np.arange(cfg.L)
        degp = np.zeros(cfg.LP, np.int64)
        degp[:cfg.L] = deg[perm]
        pos = inv[ed]
        order = np.argsort(pos, kind="stable")
        percore.append((es[order], pos[order], degp, perm))
        invs.append(inv)
    D = np.ones(cfg.TILES, np.int64)
    for es, pos, degp, perm in percore:
        D = np.maximum(D, degp.reshape(cfg.TILES, 128).max(1))
    return percore, invs, D


def _chunk_sched(cfg, D, cap_slots=64, max_tiles=6):
    cap_slots = max(cap_slots, int(D.max()))
    chunks, t = [], 0
    while t < cfg.TILES:
        nt = 1
        while (t + nt < cfg.TILES and nt < max_tiles
               and int(max(D[t:t + nt + 1])) * (nt + 1) <= cap_slots):
            nt += 1
        chunks.append((t, nt, int(max(D[t:t + nt]))))
        t += nt
    offs = np.concatenate([[0], np.cumsum([nt * dc for _, nt, dc in chunks])])
    return chunks, offs.astype(np.int64)


def _edge_layout(cfg, percore_c, chunks, offs):
    """Flat slot index (lane*S + col) per edge for one core."""
    es, pos, degp, perm = percore_c
    S = int(offs[-1])
    start = np.concatenate([[0], np.cumsum(degp)])[:-1]
    rr = np.arange(len(pos)) - start[pos]
    tile_of = pos // 128
    lane = pos % 128
    tile2col = np.zeros(cfg.TILES, np.int64)
    for ci, (t0, nt, dc) in enumerate(chunks):
        tile2col[t0:t0 + nt] = offs[ci] + np.arange(nt) * dc
    col = tile2col[tile_of] + rr
    return lane * S + col, S, tile2col


# ======================================================================
# host prep: per-edge grids
# ======================================================================
XES = 14                 # xe slot stride: [x(10), a1s_h0..h3]


def _xe_grid(cfg, x, a1s, percore_c, flat, S, tile2col):
    """xe [128, S*14] bf16: [x(10), a1s_h0..h3]; pads: a1s cols = NEG.
    Dummy slot 0 of zero-degree lanes = this core's perm-slot-0 node."""
    es, pos, degp, perm = percore_c
    g = np.zeros((128 * S, XES), np.float32)
    g[:, cfg.F:cfg.F + cfg.H1] = NEG
    g[flat, :cfg.F] = x[es]
    g[flat, cfg.F:cfg.F + cfg.H1] = a1s[es]
    dummy = np.nonzero(degp == 0)[0]
    if len(dummy):
        lanes, tiles = dummy % 128, dummy // 128
        dflat = lanes * S + tile2col[tiles]
        n0 = perm[0]
        g[dflat, :cfg.F] = x[n0]
        g[dflat, cfg.F:cfg.F + cfg.H1] = a1s[n0]
    return g.reshape(128, S * XES).astype(bf16)


GMAX = 8                 # gather group width (<=1024 idxs per dma_gather)


def _wrap16_chunks(lin2d, chunks, offs):
    """lin2d [128, S] slot-major wrap per <=GMAX-col group -> [16, 8*S] int16."""
    S = lin2d.shape[1]
    out = np.zeros((16, 8 * S), np.int16)
    for ci, (t0, nt, dc) in enumerate(chunks):
        o0, o1 = int(offs[ci]), int(offs[ci + 1])
        for g0 in range(o0, o1, GMAX):
            g1 = min(g0 + GMAX, o1)
            lin = lin2d[:, g0:g1].T.reshape(-1)   # i = (col-g0)*128 + lane
            n = len(lin)
            w = np.zeros((16, n // 16), np.int16)
            w[np.arange(n) % 16, np.arange(n) // 16] = lin
            out[:, g0 * 8:g1 * 8] = w
    return out


def _gat2_planes(cfg, percore_c, invs, c, flat, S, tile2col):
    """Gather planes for GAT2: idxa/idxb [16, 8S] i16, flags [128, S] bf16."""
    es, pos, degp, perm = percore_c
    owner = es // cfg.L
    r = np.empty_like(es)
    for oc in range(cfg.NCORE):
        m = owner == oc
        if m.any():
            r[m] = invs[oc][es[m] - oc * cfg.L]
    grow = owner * cfg.LP + r
    pair = grow // 2
    half = grow % 2
    isb = pair >= SPL

    idxa = np.zeros((128 * S,), np.int64)
    idxb = np.zeros((128 * S,), np.int64)
    flags = np.full((128 * S,), 9.0, np.float32)
    idxa[flat] = np.where(isb, 0, pair)
    idxb[flat] = np.where(isb, pair - SPL, 0)
    flags[flat] = 2 * isb.astype(np.int64) + half

    dummy = np.nonzero(degp == 0)[0]
    if len(dummy):
        lanes, tiles = dummy % 128, dummy // 128
        dflat = lanes * S + tile2col[tiles]
        dgrow = c * cfg.LP + 0                    # own perm-slot-0 node
        dpair, dhalf = dgrow // 2, dgrow % 2
        if dpair >= SPL:
            idxa[dflat] = 0
            idxb[dflat] = dpair - SPL
            flags[dflat] = 2 + dhalf
        else:
            idxa[dflat] = dpair
            idxb[dflat] = 0
            flags[dflat] = dhalf

    idxa = idxa.reshape(128, S)
    idxb = idxb.reshape(128, S)
    return (_wrap16_chunks(idxa, *_cho(cfg)), _wrap16_chunks(idxb, *_cho(cfg)),
            flags.reshape(128, S).astype(bf16))


_CHO = None


def _cho(cfg):
    return _CHO


# ======================================================================
# host prep: packing
# ======================================================================
def _pack_td(cfg, td, perm, c):
    """Tight pack: tdA [TILES, 4, 121, 128] (q0..3), tdB [TILES, 66, 128]."""
    tdp = np.zeros((cfg.LP, cfg.T, cfg.F), np.float32)
    tdp[:cfg.L] = td[c * cfg.L + perm]
    nodes = tdp.reshape(cfg.TILES, 128, cfg.T, cfg.F)
    outA = np.zeros((cfg.TILES, 4, 121, 128), np.float32)
    outB = np.zeros((cfg.TILES, 66, 128), np.float32)
    for q in range(5):
        tq, kq = cfg.TQ[q], cfg.KQ[q]
        blk = nodes[:, :, cfg.TQOFF[q]:cfg.TQOFF[q] + tq, :]
        r = np.zeros((cfg.TILES, tq, 11, 128), np.float32)
        r[:, :, :cfg.F] = blk.transpose(0, 2, 3, 1)
        r[:, :, cfg.F] = 1.0
        if q < 4:
            outA[:, q] = r.reshape(cfg.TILES, kq, 128)
        else:
            outB[:] = r.reshape(cfg.TILES, kq, 128)
    return (np.ascontiguousarray(outA).astype(bf16),
            np.ascontiguousarray(outB).astype(bf16))


def _block_diag(b, n):
    r, c = b.shape
    out = np.zeros((n * r, n * c), np.float32)
    for i in range(n):
        out[i * r:(i + 1) * r, i * c:(i + 1) * c] = b
    return out


def _prep_weights(cfg, w):
    F, H1, C1, LAT = cfg.F, cfg.H1, cfg.C1, cfg.LAT
    gW1 = w["gW1"].astype(np.float32)
    A1s = (gW1.reshape(F, H1, C1) * w["ga1_src"][None]).sum(-1)
    A1d = (gW1.reshape(F, H1, C1) * w["ga1_dst"][None]).sum(-1)
    gW2 = w["gW2"].astype(np.float32)
    A2s = (gW2.reshape(H1 * C1, 1, LAT) * w["ga2_src"][None]).sum(-1)[:, 0]
    A2d = (gW2.reshape(H1 * C1, 1, LAT) * w["ga2_dst"][None]).sum(-1)[:, 0]

    con = {}
    W1b = np.concatenate([w["tW1"], w["tb1"][None]], 0).astype(np.float32)
    con["rhs_mm1f"] = _block_diag(W1b, 11)                    # [121, 704]
    con["rhs_mm1p"] = _block_diag(W1b, 6)                     # [66, 384]

    rhs_g1 = np.zeros((4 * F, H1 * C1), np.float32)
    for h in range(H1):
        rhs_g1[h * F:(h + 1) * F, h * C1:(h + 1) * C1] = gW1[:, h * C1:(h + 1) * C1]
    con["rhs_g1"] = rhs_g1                                    # [40, 128] f32
    con["gb1bc"] = np.tile(w["gb1"][None], (128, 1))          # [128, 128] f32

    gw2ext = np.zeros((H1 * C1, 68), np.float32)
    gw2ext[:, :LAT] = gW2
    gw2ext[:, LAT] = A2s
    gw2ext[:, LAT + 1] = A2d
    con["gw2ext"] = gw2ext                                    # [128, 68] f32

    ones = np.ones(H1 * C1, np.float32)
    adj2 = float(-(ones @ A2s) - (ones @ A2d))
    g2shift = ones @ gW2
    cW1 = w["cW1"].astype(np.float32)
    con["cw1f"] = np.concatenate(
        [(w["tW2"] / 50.0) @ cW1[:LAT], cW1[LAT:]], 0)        # [128, 64] f32
    cb1p = (w["tb2"] @ cW1[:LAT]) + ((w["gb2"] - g2shift) @ cW1[LAT:]) + w["cb1"]
    con["cb1bc"] = np.tile(cb1p[None], (128, 1))              # [128, 64] f32
    con["cw2bc"] = np.tile(w["cW2"][:, 0][None], (128, 1))    # [128, 64] f32
    return con, adj2, float(w["cb2"][0]), A1s, A1d


# ======================================================================
# device builder: single program
# ======================================================================
def _mk(ap_tensor, offset, dims):
    from concourse.bass import AP
    return AP(ap_tensor, int(offset), [list(d) for d in dims])


def build_exec(cfg, chunks, offs, adj2, cb2v):
    import concourse.bacc as bacc
    import concourse.mybir as mybir
    import concourse.tile as tile
    from concourse.masks import make_identity

    fp32 = mybir.dt.float32
    b16 = mybir.dt.bfloat16
    i16 = mybir.dt.int16
    F, H1, TQ, KQ, QW = cfg.F, cfg.H1, cfg.TQ, cfg.KQ, cfg.QW
    S = int(offs[-1])
    WMAX = max(nt * dc for _, nt, dc in chunks)
    NTMAX = max(nt for _, nt, dc in chunks)

    nc = bacc.Bacc(None, target_bir_lowering=False, debug=False)
    tdA = nc.dram_tensor("tdA", [cfg.TILES * 4 * 121, 128], b16, kind="ExternalInput")
    tdB = nc.dram_tensor("tdB", [cfg.TILES * 66, 128], b16, kind="ExternalInput")
    xe = nc.dram_tensor("xe", [128, S * XES], b16, kind="ExternalInput")
    a1d_i = nc.dram_tensor("a1d_i", [128, cfg.TILES * 4], b16, kind="ExternalInput")
    idxa = nc.dram_tensor("idxa", [16, 8 * S], i16, kind="ExternalInput")
    idxb = nc.dram_tensor("idxb", [16, 8 * S], i16, kind="ExternalInput")
    flg = nc.dram_tensor("flg", [128, S], b16, kind="ExternalInput")
    c_mm1f = nc.dram_tensor("c_mm1f", [121, 704], b16, kind="ExternalInput")
    c_mm1p = nc.dram_tensor("c_mm1p", [66, 384], b16, kind="ExternalInput")
    c_g1 = nc.dram_tensor("c_g1", [40, 128], fp32, kind="ExternalInput")
    c_gb1 = nc.dram_tensor("c_gb1", [128, 128], fp32, kind="ExternalInput")
    c_g2e = nc.dram_tensor("c_g2e", [128, 68], fp32, kind="ExternalInput")
    c_cw1 = nc.dram_tensor("c_cw1", [128, 64], fp32, kind="ExternalInput")
    c_cb1 = nc.dram_tensor("c_cb1", [128, 64], fp32, kind="ExternalInput")
    c_cw2 = nc.dram_tensor("c_cw2", [128, 64], fp32, kind="ExternalInput")
    o_p = nc.dram_tensor("o_p", [128, cfg.TILES], fp32, kind="ExternalOutput")

    with tile.TileContext(nc) as tc:
        with (
            tc.tile_pool(name="dram", bufs=1, space="DRAM") as dram,
            tc.tile_pool(name="const", bufs=1) as cp,
            tc.tile_pool(name="per", bufs=1) as per,
            tc.tile_pool(name="tds", bufs=3) as tds,
            tc.tile_pool(name="qps", bufs=2, space="PSUM") as qps,
            tc.tile_pool(name="gps", bufs=2, space="PSUM") as gps,
            tc.tile_pool(name="ev", bufs=2) as ev,
            tc.tile_pool(name="xep", bufs=2) as xep,
            tc.tile_pool(name="ed", bufs=2) as ed,
            tc.tile_pool(name="big", bufs=1) as big,
            tc.tile_pool(name="tl", bufs=2) as tl,
            tc.tile_pool(name="gth", bufs=1) as gth,
            tc.tile_pool(name="idxp", bufs=2) as idxp,
            tc.tile_pool(name="mrg", bufs=1) as mrg,
        ):
            g2loc = dram.tile([cfg.PAIRS, 256], b16)
            g2all = dram.tile([cfg.NCORE * cfg.PAIRS, 256], b16)

            ident = cp.tile([128, 128], fp32)
            make_identity(nc, ident[:])
            mm1f = cp.tile([121, 704], b16)
            nc.sync.dma_start(out=mm1f[:], in_=c_mm1f[:])
            mm1p = cp.tile([66, 384], b16)
            nc.sync.dma_start(out=mm1p[:], in_=c_mm1p[:])
            g1W = cp.tile([40, 128], fp32)
            nc.sync.dma_start(out=g1W[:], in_=c_g1[:])
            gb1bc = cp.tile([128, 128], fp32)
            nc.sync.dma_start(out=gb1bc[:], in_=c_gb1[:])
            g2eW = cp.tile([128, 68], fp32)
            nc.sync.dma_start(out=g2eW[:], in_=c_g2e[:])
            cw1 = cp.tile([128, 64], fp32)
            nc.sync.dma_start(out=cw1[:], in_=c_cw1[:])
            cb1 = cp.tile([128, 64], fp32)
            nc.sync.dma_start(out=cb1[:], in_=c_cb1[:])
            cw2 = cp.tile([128, 64], fp32)
            nc.sync.dma_start(out=cw2[:], in_=c_cw2[:])
            adj_t = cp.tile([128, 1], fp32)
            nc.vector.memset(adj_t[:], float(adj2))
            cb2_t = cp.tile([128, 1], fp32)
            nc.vector.memset(cb2_t[:], float(cb2v))
            a1d_all = per.tile([128, cfg.TILES * 4], b16)
            nc.sync.dma_start(out=a1d_all[:], in_=a1d_i[:])
            flA = per.tile([128, S], b16)
            nc.sync.dma_start(out=flA[:], in_=flg[:])

            spre = per.tile([128, cfg.TILES * 64], fp32)
            a2d_all = per.tile([128, cfg.TILES], fp32)
            prob = per.tile([128, cfg.TILES], fp32)

            # ---- GAT1 per chunk -> g2loc pair rows ----
            for ci, (t0, nt, dc) in enumerate(chunks):
                W = nt * dc
                xeb = xep.tile([128, WMAX * XES], b16, tag="xeb")
                nc.sync.dma_start(
                    out=xeb[:, 0:W * XES],
                    in_=xe[:, int(offs[ci]) * XES:(int(offs[ci]) + W) * XES])
                # esum layout (h, t, j): xe a1s col + a1d_dst
                esum = ed.tile([128, WMAX * 4], b16, tag="esum")
                for h in range(4):
                    nc.vector.tensor_tensor(
                        out=esum[:, h * W:(h + 1) * W],
                        in0=_mk(xeb.tensor, F + h,
                                [xeb[:].ap[0], [XES * dc, nt], [XES, dc]]),
                        in1=_mk(a1d_all.tensor, 4 * t0 + h,
                                [a1d_all[:].ap[0], [4, nt], [0, dc]]),
                        op=mybir.AluOpType.add)
                lr = ed.tile([128, WMAX * 4], b16, tag="lr")
                nc.vector.scalar_tensor_tensor(
                    out=lr[:, 0:W * 4], in0=esum[:, 0:W * 4], scalar=0.2,
                    in1=esum[:, 0:W * 4],
                    op0=mybir.AluOpType.mult, op1=mybir.AluOpType.max)
                wv = ed.tile([128, WMAX * 4], b16, tag="wv")
                nc.scalar.activation(out=wv[:, 0:W * 4], in_=lr[:, 0:W * 4],
                                     func=mybir.ActivationFunctionType.Exp)
                den = ed.tile([128, NTMAX * 4], fp32, tag="den")
                nc.vector.tensor_reduce(
                    out=den[:, 0:nt * 4],
                    in_=_mk(wv.tensor, 0, [wv[:].ap[0], [dc, 4 * nt], [1, dc]]),
                    axis=mybir.AxisListType.X, op=mybir.AluOpType.add)
                rec = ed.tile([128, NTMAX * 4], fp32, tag="rec")
                nc.vector.reciprocal(out=rec[:, 0:nt * 4], in_=den[:, 0:nt * 4])
                tmpx = ed.tile([128, WMAX * 10], b16, tag="tmpx")
                xaggr = ed.tile([128, NTMAX * 40], fp32, tag="xaggr")
                for h in range(4):
                    nc.vector.tensor_tensor(
                        out=tmpx[:, 0:W * 10],
                        in0=_mk(xeb.tensor, 0,
                                [xeb[:].ap[0], [XES * dc, nt], [XES, dc], [1, 10]]),
                        in1=_mk(wv.tensor, h * W,
                                [wv[:].ap[0], [dc, nt], [1, dc], [0, 10]]),
                        op=mybir.AluOpType.mult)
                    nc.vector.tensor_reduce(
                        out=_mk(xaggr.tensor, 10 * h,
                                [xaggr[:].ap[0], [40, nt], [1, 10]]),
                        in_=_mk(tmpx.tensor, 0,
                                [tmpx[:].ap[0], [10 * dc, nt], [1, 10], [10, dc]]),
                        axis=mybir.AxisListType.X, op=mybir.AluOpType.add)
                xagg = ed.tile([128, NTMAX * 40], fp32, tag="xagg")
                nc.vector.tensor_tensor(
                    out=xagg[:, 0:nt * 40],
                    in0=xaggr[:, 0:nt * 40],
                    in1=_mk(rec.tensor, 0,
                            [rec[:].ap[0], [1, nt], [nt, 4], [0, 10]]),
                    op=mybir.AluOpType.mult)
                for ti in range(nt):
                    t = t0 + ti
                    ps1 = gps.tile([128, 128], fp32, tag="g")
                    nc.tensor.transpose(out=ps1[0:40, :],
                                        in_=xagg[:, 40 * ti:40 * (ti + 1)],
                                        identity=ident[:])
                    stag = tl.tile([40, 128], fp32, tag="stag1")
                    nc.vector.tensor_copy(out=stag[:], in_=ps1[0:40, :])
                    out1 = gps.tile([128, 128], fp32, tag="g")
                    nc.tensor.matmul(out1[:], lhsT=stag[:], rhs=g1W[:],
                                     start=True, stop=True)
                    y1 = tl.tile([128, 128], fp32, tag="y1")
                    nc.vector.tensor_add(out=y1[:], in0=out1[:], in1=gb1bc[:])
                    mn = tl.tile([128, 128], fp32, tag="mn")
                    nc.vector.tensor_scalar_min(out=mn[:], in0=y1[:], scalar1=0.0)
                    ex = tl.tile([128, 128], fp32, tag="ex")
                    nc.scalar.activation(out=ex[:], in_=mn[:],
                                         func=mybir.ActivationFunctionType.Exp)
                    hs = tl.tile([128, 128], fp32, tag="hs")
                    nc.vector.scalar_tensor_tensor(
                        out=hs[:], in0=y1[:], scalar=0.0, in1=ex[:],
                        op0=mybir.AluOpType.max, op1=mybir.AluOpType.add)
                    ps2 = gps.tile([128, 128], fp32, tag="g")
                    nc.tensor.transpose(out=ps2[:], in_=hs[:], identity=ident[:])
                    hT = tl.tile([128, 128], fp32, tag="hT")
                    nc.vector.tensor_copy(out=hT[:], in_=ps2[:])
                    g2p = gps.tile([128, 128], fp32, tag="g")
                    nc.tensor.matmul(g2p[:, 0:68], lhsT=hT[:], rhs=g2eW[:],
                                     start=True, stop=True)
                    g2s = tl.tile([128, 68], b16, tag="g2s")
                    nc.vector.tensor_copy(out=g2s[:], in_=g2p[:, 0:68])
                    # pair-row write: row = 64t + p//2, half = p%2
                    nc.sync.dma_start(
                        out=_mk(g2loc.tensor, t * 64 * 256,
                                [[256, 64], [68, 2], [1, 68]]),
                        in_=g2s[:])
                    nc.scalar.activation(out=a2d_all[:, t:t + 1],
                                         in_=g2p[:, 65:66],
                                         func=mybir.ActivationFunctionType.Identity,
                                         bias=adj_t[:])

            # ---- halo exchange ----
            nc.gpsimd.collective_compute(
                "AllGather", mybir.AluOpType.bypass,
                replica_groups=[list(range(cfg.NCORE))],
                ins=[g2loc[:]], outs=[g2all[:]])

            # ---- temporal encoder: spre = sum_t relu(z_t) ----
            for t in range(cfg.TILES):
                st = tds.tile([128, 640], b16)
                nc.sync.dma_start(
                    out=_mk(st.tensor, 0, [[st[:].ap[0][0], 121], [128, 4], [1, 128]]),
                    in_=_mk(tdA, t * 4 * 121 * 128,
                            [[128, 121], [121 * 128, 4], [1, 128]]))
                nc.sync.dma_start(
                    out=_mk(st.tensor, 512, [[st[:].ap[0][0], 66], [1, 128]]),
                    in_=_mk(tdB, t * 66 * 128, [[128, 66], [1, 128]]))
                stg = ev.tile([128, 3200], b16, tag="evs")
                for q in range(5):
                    kq, w = KQ[q], QW[q]
                    rhs = mm1f if q < 4 else mm1p
                    ps = qps.tile([128, 704], fp32, tag="qtile")
                    lhsT = st[0:kq, 128 * q:128 * (q + 1)]
                    for c0 in range(0, w, 512):
                        c1 = min(c0 + 512, w)
                        nc.tensor.matmul(ps[:, c0:c1], lhsT=lhsT,
                                         rhs=rhs[0:kq, c0:c1], start=True, stop=True)
                    nc.scalar.activation(out=stg[:, 704 * q:704 * q + w],
                                         in_=ps[:, 0:w],
                                         func=mybir.ActivationFunctionType.Relu)
                nc.vector.tensor_reduce(
                    out=spre[:, 64 * t:64 * (t + 1)],
                    in_=_mk(stg.tensor, 0, [stg[:].ap[0], [1, 64], [64, 50]]),
                    axis=mybir.AxisListType.X, op=mybir.AluOpType.add)

            # ---- GAT2 per chunk (gather + segment softmax) + classifier ----
            for ci, (t0, nt, dc) in enumerate(chunks):
                W = nt * dc
                o0 = int(offs[ci])
                ia = idxp.tile([128, WMAX * 8], i16, tag="ia")
                nc.sync.dma_start(
                    out=ia[:, 0:W * 8],
                    in_=_mk(idxa, o0 * 8, [[0, 8], [8 * S, 16], [1, W * 8]]))
                ib = idxp.tile([128, WMAX * 8], i16, tag="ib")
                nc.sync.dma_start(
                    out=ib[:, 0:W * 8],
                    in_=_mk(idxb, o0 * 8, [[0, 8], [8 * S, 16], [1, W * 8]]))
                gA = gth.tile([128, WMAX * 256], b16, tag="gA")
                gB = gth.tile([128, WMAX * 256], b16, tag="gB")
                for r0 in range(0, W, GMAX):
                    r1 = min(r0 + GMAX, W)
                    gw = r1 - r0
                    nc.gpsimd.dma_gather(
                        _mk(gA.tensor, r0 * 256, [gA[:].ap[0], [256, gw], [1, 256]]),
                        _mk(g2all.tensor, 0, [[256, SPL], [1, 256]]),
                        ia[:, r0 * 8:r1 * 8], gw * 128, gw * 128, 256)
                    nc.gpsimd.dma_gather(
                        _mk(gB.tensor, r0 * 256, [gB[:].ap[0], [256, gw], [1, 256]]),
                        _mk(g2all.tensor, SPL * 256,
                            [[256, cfg.NR2 - SPL], [1, 256]]),
                        ib[:, r0 * 8:r1 * 8], gw * 128, gw * 128, 256)
                m = []
                for k in range(4):
                    mk_ = mrg.tile([128, WMAX], b16, tag=f"m{k}")
                    nc.vector.tensor_scalar(
                        out=mk_[:, 0:W], in0=flA[:, o0:o0 + W], scalar1=float(k),
                        scalar2=None, op0=mybir.AluOpType.is_equal)
                    m.append(mk_)
                vm = mrg.tile([128, WMAX], b16, tag="vm")
                nc.vector.tensor_scalar(
                    out=vm[:, 0:W], in0=flA[:, o0:o0 + W], scalar1=4.0,
                    scalar2=None, op0=mybir.AluOpType.is_lt)
                ge68 = mrg.tile([128, WMAX * 68], b16, tag="ge68")
                t1 = mrg.tile([128, WMAX * 68], b16, tag="t1")
                t2 = mrg.tile([128, WMAX * 68], b16, tag="t2")
                for gt, m0, m1, first in ((gA, m[0], m[1], True),
                                          (gB, m[2], m[3], False)):
                    nc.vector.tensor_tensor(
                        out=t1[:, 0:W * 68],
                        in0=_mk(gt.tensor, 0, [gt[:].ap[0], [256, W], [1, 68]]),
                        in1=_mk(m0.tensor, 0, [m0[:].ap[0], [1, W], [0, 68]]),
                        op=mybir.AluOpType.mult)
                    nc.vector.tensor_tensor(
                        out=t2[:, 0:W * 68],
                        in0=_mk(gt.tensor, 68, [gt[:].ap[0], [256, W], [1, 68]]),
                        in1=_mk(m1.tensor, 0, [m1[:].ap[0], [1, W], [0, 68]]),
                        op=mybir.AluOpType.mult)
                    if first:
                        nc.vector.tensor_add(out=ge68[:, 0:W * 68],
                                             in0=t1[:, 0:W * 68], in1=t2[:, 0:W * 68])
                    else:
                        nc.vector.tensor_add(out=t1[:, 0:W * 68],
                                             in0=t1[:, 0:W * 68], in1=t2[:, 0:W * 68])
                        nc.vector.tensor_add(out=ge68[:, 0:W * 68],
                                             in0=ge68[:, 0:W * 68], in1=t1[:, 0:W * 68])
                # segment softmax over incoming edges
                es2 = ed.tile([128, WMAX], b16, tag="es2")
                nc.vector.tensor_tensor(
                    out=es2[:, 0:W],
                    in0=_mk(ge68.tensor, 64, [ge68[:].ap[0], [68, W]]),
                    in1=_mk(a2d_all.tensor, t0, [a2d_all[:].ap[0], [1, nt], [0, dc]]),
                    op=mybir.AluOpType.add)
                lr2 = ed.tile([128, WMAX], b16, tag="lr2")
                nc.vector.scalar_tensor_tensor(
                    out=lr2[:, 0:W], in0=es2[:, 0:W], scalar=0.2, in1=es2[:, 0:W],
                    op0=mybir.AluOpType.mult, op1=mybir.AluOpType.max)
                w2 = ed.tile([128, WMAX], b16, tag="w2")
                nc.scalar.activation(out=w2[:, 0:W], in_=lr2[:, 0:W],
                                     func=mybir.ActivationFunctionType.Exp)
                w2v = ed.tile([128, WMAX], b16, tag="w2v")
                nc.vector.tensor_tensor(out=w2v[:, 0:W], in0=w2[:, 0:W],
                                        in1=vm[:, 0:W], op=mybir.AluOpType.mult)
                den2 = ed.tile([128, NTMAX], fp32, tag="den2")
                nc.vector.tensor_reduce(
                    out=den2[:, 0:nt],
                    in_=_mk(w2v.tensor, 0, [w2v[:].ap[0], [dc, nt], [1, dc]]),
                    axis=mybir.AxisListType.X, op=mybir.AluOpType.add)
                rec2 = ed.tile([128, NTMAX], fp32, tag="rec2")
                nc.vector.reciprocal(out=rec2[:, 0:nt], in_=den2[:, 0:nt])
                w2n = ed.tile([128, WMAX], b16, tag="w2n")
                nc.vector.tensor_tensor(
                    out=w2n[:, 0:W], in0=w2v[:, 0:W],
                    in1=_mk(rec2.tensor, 0, [rec2[:].ap[0], [1, nt], [0, dc]]),
                    op=mybir.AluOpType.mult)
                tmp2 = big.tile([128, WMAX * 64], b16, tag="tmp2")
                nc.vector.tensor_tensor(
                    out=tmp2[:, 0:W * 64],
                    in0=_mk(ge68.tensor, 0, [ge68[:].ap[0], [68, W], [1, 64]]),
                    in1=_mk(w2n.tensor, 0, [w2n[:].ap[0], [1, W], [0, 64]]),
                    op=mybir.AluOpType.mult)
                out2 = big.tile([128, NTMAX * 64], fp32, tag="out2")
                nc.vector.tensor_reduce(
                    out=out2[:, 0:nt * 64],
                    in_=_mk(tmp2.tensor, 0,
                            [tmp2[:].ap[0], [64 * dc, nt], [1, 64], [64, dc]]),
                    axis=mybir.AxisListType.X, op=mybir.AluOpType.add)
                for ti in range(nt):
                    t = t0 + ti
                    ps1 = gps.tile([128, 128], fp32, tag="g")
                    nc.tensor.transpose(out=ps1[0:64, :],
                                        in_=spre[:, 64 * t:64 * (t + 1)],
                                        identity=ident[:])
                    stag = tl.tile([128, 128], fp32, tag="stag2")
                    nc.vector.tensor_copy(out=stag[0:64, :], in_=ps1[0:64, :])
                    ps2 = gps.tile([128, 128], fp32, tag="g")
                    nc.tensor.transpose(out=ps2[0:64, :],
                                        in_=out2[:, 64 * ti:64 * (ti + 1)],
                                        identity=ident[:])
                    nc.vector.tensor_copy(out=stag[64:128, :], in_=ps2[0:64, :])
                    z1 = gps.tile([128, 64], fp32, tag="g")
                    nc.tensor.matmul(z1[:], lhsT=stag[:], rhs=cw1[:],
                                     start=True, stop=True)
                    y = tl.tile([128, 64], fp32, tag="y")
                    nc.vector.tensor_add(out=y[:], in0=z1[:], in1=cb1[:])
                    nc.vector.tensor_scalar_max(out=y[:], in0=y[:], scalar1=0.0)
                    zt = tl.tile([128, 64], fp32, tag="zt")
                    nc.vector.tensor_tensor(out=zt[:], in0=y[:], in1=cw2[:],
                                            op=mybir.AluOpType.mult)
                    zz = tl.tile([128, 1], fp32, tag="zz")
                    nc.vector.tensor_reduce(out=zz[:], in_=zt[:],
                                            axis=mybir.AxisListType.X,
                                            op=mybir.AluOpType.add)
                    nc.scalar.activation(out=prob[:, t:t + 1], in_=zz[:],
                                         func=mybir.ActivationFunctionType.Sigmoid,
                                         bias=cb2_t[:])
            nc.sync.dma_start(out=o_p[:], in_=prob[:])
    nc.finalize()
    return nc


# ======================================================================
# top level
# ======================================================================
def _run(nc, in_maps, ncore):
    from concourse.bass_utils import run_bass_kernel_spmd
    return run_bass_kernel_spmd(nc, in_maps, core_ids=list(range(ncore))).results


def kernel(temporal_data, x, edge_index, tW1, tb1, tW2, tb2,
           gW1, ga1_src, ga1_dst, gb1, gW2, ga2_src, ga2_dst, gb2,
           cW1, cb1, cW2, cb2, _cfg=None, _runner=None):
    global _CHO
    cfg = _cfg or CFG
    x = np.asarray(x, np.float32)
    td = np.asarray(temporal_data, np.float32)
    w = dict(tW1=np.asarray(tW1, np.float32), tb1=np.asarray(tb1, np.float32),
             tW2=np.asarray(tW2, np.float32), tb2=np.asarray(tb2, np.float32),
             gW1=np.asarray(gW1, np.float32), ga1_src=np.asarray(ga1_src, np.float32),
             ga1_dst=np.asarray(ga1_dst, np.float32), gb1=np.asarray(gb1, np.float32),
             gW2=np.asarray(gW2, np.float32), ga2_src=np.asarray(ga2_src, np.float32),
             ga2_dst=np.asarray(ga2_dst, np.float32), gb2=np.asarray(gb2, np.float32),
             cW1=np.asarray(cW1, np.float32), cb1=np.asarray(cb1, np.float32),
             cW2=np.asarray(cW2, np.float32), cb2=np.asarray(cb2, np.float32))

    percore, invs, D = _prep_graph(cfg, edge_index)
    chunks, offs = _chunk_sched(cfg, D)
    _CHO = (chunks, offs)
    con, adj2, cb2v, A1s, A1d = _prep_weights(cfg, w)
    a1s_all = x @ A1s                       # [N, 4]
    a1d_vals = x @ A1d                      # [N, 4]

    ins = []
    for c in range(cfg.NCORE):
        flat, S, tile2col = _edge_layout(cfg, percore[c], chunks, offs)
        perm = percore[c][3]
        a1dg = np.zeros((cfg.LP, 4), np.float32)
        a1dg[:cfg.L] = a1d_vals[c * cfg.L + perm]
        ia, ib, fl = _gat2_planes(cfg, percore[c], invs, c, flat, S, tile2col)
        tdA, tdB = _pack_td(cfg, td, perm, c)
        ins.append({
            "tdA": tdA.reshape(cfg.TILES * 4 * 121, 128),
            "tdB": tdB.reshape(cfg.TILES * 66, 128),
            "xe": _xe_grid(cfg, x, a1s_all, percore[c], flat, S, tile2col),
            "a1d_i": a1dg.reshape(cfg.TILES, 128, 4).transpose(1, 0, 2)
                         .reshape(128, cfg.TILES * 4).astype(bf16),
            "idxa": ia, "idxb": ib, "flg": fl,
            "c_mm1f": con["rhs_mm1f"].astype(bf16),
            "c_mm1p": con["rhs_mm1p"].astype(bf16),
            "c_g1": con["rhs_g1"].astype(np.float32),
            "c_gb1": con["gb1bc"].astype(np.float32),
            "c_g2e": con["gw2ext"].astype(np.float32),
            "c_cw1": con["cw1f"].astype(np.float32),
            "c_cb1": con["cb1bc"].astype(np.float32),
            "c_cw2": con["cw2bc"].astype(np.float32),
        })

    nc = build_exec(cfg, chunks, offs, adj2, cb2v)
    runner = _runner or _run
    res = runner(nc, ins, cfg.NCORE)

    out = np.zeros((cfg.N, 1), np.float32)
    for c in range(cfg.NCORE):
        p = np.asarray(res[c]["o_p"])           # [128, TILES] (lane, tile)
        pl = p.T.reshape(cfg.LP)                # perm position -> prob
        out[c * cfg.L:(c + 1) * cfg.L, 0] = pl[invs[c]]
    return out
